# revision 1
# baseline (speedup 1.0000x reference)
"""Trainium2 Bass kernel for nn_MultiHeadedAttention_41566693491186.

Three dual-score MHAs over the streams packed in x[:, :, 0:3, :], with shared
Wq/Wk/Wv/Wo. Data-parallel over batch B=8: one batch element per NeuronCore.

Per-core plan (all matmuls float32r, ~tf32 precision, 1 cyc/row at N>=512):
  P0  load x, PE-transpose each stream to xT[s] = x_s^T [D, L], spill to DRAM
  P1  projections (interleaved with attention below):
        qT[s] = (x_s @ Wq)^T, kT[s] = (x_s @ Wk)^T   (W-stationary, out [j, L])
        v[s]  =  x_s @ Wv                            (x-stationary, out [L, j])
      spilled to DRAM; v in an interleaved [64 data | 1 ones] per-head layout
      so the PV matmul's lhsT picks up a ones column that produces the softmax
      denominators as PSUM row 64 for free.
  P2  per (mha, head): S^T = kcat^T-chunks x qcat -> exp (ACT, scale=1/16)
      -> P^T in SBUF -> PV accumulate o^T[d, q] + sums row.  Softmax denom:
      recip = exp(-ln(sums)) on ACT rows, broadcast to 64 partitions by
      doubling SBUF->SBUF DMAs, normalize with one DVE mul into OT.
  P3  out = OT^T @ Wo + bo  (OT-stationary, out [q, d_model]) -> DRAM.

The attention inner loop is ACT(exp)-bound while projections are PE-bound, so
the program emits them interleaved (generator round-robin) to keep the PE
saturated and HAM-warm.
"""

import sys

if "/opt/trn_rl_repo" not in sys.path:
    sys.path.insert(0, "/opt/trn_rl_repo")

import numpy as np

B, L, D = 8, 1024, 1024
H, DH = 16, 64
NCH = 8            # 128-sized chunks along D or L
SCALE = 0.0625     # (1/sqrt(64)) * 0.5
N_CORES = 8
# mha m reads (A, B, V) streams: q1/k1 from A, q2/k2 from B, v from V
MHA_STREAMS = ((1, 2, 0), (0, 2, 1), (0, 1, 2))

_CACHE = {}


def _split_excess_waits(nc, max_waits=1):
    """Stock neuronxcc walrus rejects instructions carrying more than
    `max_waits` semaphore waits; move excess onto same-engine NOPs."""
    import concourse.mybir as mybir

    for f in nc.m.functions:
        for bb in f.blocks:
            out = []
            changed = False
            for inst in bb.instructions:
                si = inst.sync_info
                waits = list(si.on_wait) if (si is not None and si.on_wait) else []
                if len(waits) > max_waits:
                    extra, keep = waits[:-max_waits], waits[-max_waits:]
                    k = 0
                    while extra:
                        chunk, extra = extra[:max_waits], extra[max_waits:]
                        nop = mybir.InstNoOp(
                            name=f"{inst.name}-ws{k}",
                            engine=inst.engine,
                            sync_info=mybir.SyncInfo(on_wait=chunk, on_update=[]),
                        )
                        out.append(nop)
                        k += 1
                    inst.sync_info = mybir.SyncInfo(
                        on_wait=keep,
                        on_update=list(si.on_update) if si.on_update else [],
                    )
                    changed = True
                out.append(inst)
            if changed:
                bb.instructions = out


def _interleave(*seqs):
    """Proportional merge of thunk lists, preserving within-list order."""
    items = []
    for si, seq in enumerate(seqs):
        n = len(seq)
        for i, thunk in enumerate(seq):
            items.append(((i + 0.5) / n, si, i, thunk))
    for _, _, _, t in sorted(items, key=lambda z: (z[0], z[1], z[2])):
        t()


def _build_program(repeat=1):
    import concourse.bass as bass
    import concourse.mybir as mybir
    import concourse.tile as tile
    from concourse.masks import make_identity

    f32 = mybir.dt.float32
    f32r = mybir.dt.float32r
    AF = mybir.ActivationFunctionType

    nc = bass.Bass("TRN2", target_bir_lowering=False, debug=False)

    x = nc.declare_dram_parameter("x", [L, 3, D], f32, isOutput=False)
    Wq = nc.declare_dram_parameter("Wq", [D, D], f32r, isOutput=False)
    Wk = nc.declare_dram_parameter("Wk", [D, D], f32r, isOutput=False)
    Wv = nc.declare_dram_parameter("Wv", [D, D], f32r, isOutput=False)
    Wo = nc.declare_dram_parameter("Wo", [D, D], f32r, isOutput=False)
    bq = nc.declare_dram_parameter("bq", [D], f32, isOutput=False)
    bk = nc.declare_dram_parameter("bk", [D], f32, isOutput=False)
    bv = nc.declare_dram_parameter("bv", [D], f32, isOutput=False)
    bo = nc.declare_dram_parameter("bo", [D], f32, isOutput=False)
    out = nc.declare_dram_parameter("out", [L, 3, D], f32, isOutput=True)

    # internal DRAM spill
    qT_d = [nc.dram_tensor(f"qT{s}", [D, L], f32r) for s in range(3)]
    kT_d = [nc.dram_tensor(f"kT{s}", [D, L], f32r) for s in range(3)]
    # v: head h data at cols 65h..65h+64, ones column at 65h+64
    v_d = [nc.dram_tensor(f"v{s}", [L, H * 65], f32r) for s in range(3)]

    with tile.TileContext(nc) as tc:
        cstack = []
        cp = tc.alloc_tile_pool(name="const", bufs=1)
        psum = tc.alloc_tile_pool(name="psum", bufs=1, space="PSUM")
        xts = tc.alloc_tile_pool(name="xts", bufs=3)
        cstack += [cp, psum, xts]

        cmisc = cp.tile([128, 208], f32, tag="cmisc", name="cmisc")
        ident = cmisc[:, 0:128]
        ones64 = cmisc[:, 128:192]
        ones16 = cmisc[:, 128:144]
        bq_t = cmisc[:, 192:200]
        bk_t = cmisc[:, 200:208]
        make_identity(nc, ident)
        nc.gpsimd.memset(ones64, 1.0)
        nc.sync.dma_start(out=bq_t, in_=bq.rearrange("(c p) -> p c", p=128))
        nc.sync.dma_start(out=bk_t, in_=bk.rearrange("(c p) -> p c", p=128))

        # ---------------- P0: load + transpose x (block lists) ----------------
        # xT tiles are built in xts-pool slots and handed directly to the
        # first projection groups; only streams 1,2 spill to DRAM for the
        # later v-projection reloads.
        ldp = tc.alloc_tile_pool(name="p0", bufs=2)
        cstack.append(ldp)
        xt_tiles = {}

        def p0_blocks(s):
            def start():
                xt_tiles[s] = xts.tile([128, NCH * L], f32r, tag="xts",
                                       name="xts")
            def lcblk(lc):
                xt = xt_tiles[s]
                xin = ldp.tile([128, D], f32, tag="xin", name="xin")
                nc.sync.dma_start(
                    out=xin[:], in_=x[128 * lc:128 * (lc + 1), s, :])
                for g in range(2):
                    tp = psum.tile([128, 512], f32, tag="fine", name="tp",
                                   bufs=4)
                    for i in range(4):
                        c = 4 * g + i
                        nc.tensor.transpose(
                            tp[:, 128 * i:128 * (i + 1)],
                            xin[:, 128 * c:128 * (c + 1)],
                            ident)
                    dst = xt[:].rearrange(
                        "p (c l) -> p c l", l=L)[:, 4 * g:4 * g + 4,
                                                 128 * lc:128 * (lc + 1)]
                    tsrc = tp[:].rearrange("p (c l) -> p c l", l=128)
                    nc.vector.tensor_copy(dst, tsrc)
            return [start] + [lambda lc=lc: lcblk(lc) for lc in range(NCH)]

        # ---------------- shared pools for P1/P2/P3 ----------------
        wrp = tc.alloc_tile_pool(name="wrp", bufs=1)
        stp = tc.alloc_tile_pool(name="stp", bufs=5)
        qkp = tc.alloc_tile_pool(name="qkp", bufs=3)
        ptp = tc.alloc_tile_pool(name="ptp", bufs=3)
        rbp = tc.alloc_tile_pool(name="rbp", bufs=2)
        cstack += [wrp, stp, qkp, ptp, rbp]

        def load_w(Wsrc):
            w_t = wrp.tile([128, NCH * D], f32r, tag="W", name="W")
            d3 = w_t[:].rearrange("p (c d) -> p c d", d=D)
            s3 = Wsrc.rearrange("(c p) d -> p c d", p=128)
            nc.sync.dma_start(out=d3[:, 0:4, :], in_=s3[:, 0:4, :])
            nc.scalar.dma_start(out=d3[:, 4:8, :], in_=s3[:, 4:8, :])
            return w_t

        def proj_qk_blocks(w_t, b_t, s, outd, xt):
            # out [j, L] = (x_s @ W)^T, one block per jc
            def block(jc):
                def run():
                    st = stp.tile([128, L], f32r, tag="st", name="st")
                    ps = [psum.tile([128, 512], f32, tag="fine",
                                    name="pp", bufs=4) for _ in range(2)]
                    for c in range(NCH):
                        for lh in range(2):
                            nc.tensor.matmul(
                                ps[lh][:],
                                lhsT=w_t[:, D * c + 128 * jc:D * c + 128 * (jc + 1)],
                                rhs=xt[:, L * c + 512 * lh:L * c + 512 * (lh + 1)],
                                start=(c == 0), stop=(c == NCH - 1))
                    for lh in range(2):
                        nc.vector.tensor_scalar_add(
                            st[:, 512 * lh:512 * (lh + 1)], ps[lh][:],
                            b_t[:, jc:jc + 1])
                    nc.scalar.dma_start(
                        out=outd[s][128 * jc:128 * (jc + 1), :], in_=st[:])
                return run
            return [block(jc) for jc in range(NCH)]

        def proj_v_blocks(w_t, s, xt):
            # out [L, j] with interleaved ones, one block per lc
            def block(lc):
                def run():
                    ps = [psum.tile([128, 512], f32, tag="fine",
                                    name="pp", bufs=4) for _ in range(2)]
                    for c in range(NCH):
                        for jh in range(2):
                            nc.tensor.matmul(
                                ps[jh][:],
                                lhsT=xt[:, L * c + 128 * lc:L * c + 128 * (lc + 1)],
                                rhs=w_t[:, D * c + 512 * jh:D * c + 512 * (jh + 1)],
                                start=(c == 0), stop=(c == NCH - 1))
                    for jh in range(2):
                        vst = stp.tile([128, 8 * 65], f32r, tag="st", name="vst")
                        r = vst[:].rearrange("p (h w) -> p h w", w=65)
                        q3 = ps[jh][:].rearrange(
                            "p (h w) -> p h w", w=64)
                        nc.vector.tensor_copy(r[:, :, 0:64], q3)
                        nc.vector.tensor_copy(
                            r[:, :, 64:65].squeeze(2), ones16[:, 0:8])
                        nc.scalar.dma_start(
                            out=v_d[s][128 * lc:128 * (lc + 1),
                                       8 * 65 * jh:8 * 65 * (jh + 1)],
                            in_=vst[:])
                return run
            return [block(lc) for lc in range(NCH)]

        def attention_blocks(m, OT):
            sa, sb, sv = MHA_STREAMS[m]
            pend = {}

            def finalize(h, oc):
                # softmax denominators: recip = exp(-ln(sums)); broadcast to
                # 64 partitions with an exact fp32 K=1 outer product (ones x
                # recip row) and normalize straight out of PSUM
                rb = rbp.tile([64, L], f32, tag="rb", name="rb")
                nc.scalar.activation(rb[32:33, :], oc[64:65, :], AF.Ln)
                nc.scalar.activation(rb[0:1, :], rb[32:33, :],
                                     AF.Exp, scale=-1.0)
                po, co = 64 * (h % 2), (h // 2) * L
                for qh in range(2):
                    rb_ps = psum.tile([64, 512], f32, tag="fine",
                                      name="rb_ps", bufs=4)
                    nc.tensor.matmul(
                        rb_ps[:], lhsT=cmisc[0:1, 128:192],
                        rhs=rb[0:1, 512 * qh:512 * (qh + 1)],
                        start=True, stop=True)
                    nc.vector.tensor_mul(
                        OT[po:po + 64, co + 512 * qh:co + 512 * (qh + 1)],
                        oc[0:64, 512 * qh:512 * (qh + 1)], rb_ps[:])

            def head(h):
                def run():
                    qcat = qkp.tile([128, L], f32r, tag="qcat", name="qcat")
                    kcat = qkp.tile([128, L], f32r, tag="kcat", name="kcat")
                    nc.sync.dma_start(
                        out=qcat[0:64, :], in_=qT_d[sa][64 * h:64 * h + 64, :])
                    nc.sync.dma_start(
                        out=qcat[64:128, :], in_=qT_d[sb][64 * h:64 * h + 64, :])
                    nc.sync.dma_start(
                        out=kcat[0:64, :], in_=kT_d[sa][64 * h:64 * h + 64, :])
                    nc.sync.dma_start(
                        out=kcat[64:128, :], in_=kT_d[sb][64 * h:64 * h + 64, :])
                    vext = qkp.tile([128, NCH * 65], f32r, tag="vext", name="vext", bufs=2)
                    vsrc = v_d[sv].rearrange("(c p) w -> p c w", p=128)
                    nc.sync.dma_start(
                        out=vext[:].rearrange("p (c w) -> p c w", w=65),
                        in_=vsrc[:, :, 65 * h:65 * (h + 1)])

                    o_ps = [psum.tile([65, 512], f32, tag="fine",
                                      name="ops", bufs=4) for _ in range(2)]
                    for c in range(NCH):
                        s_ps = psum.tile([128, L], f32, tag="scr", name="scr", bufs=2)
                        for qh in range(2):
                            nc.tensor.matmul(
                                s_ps[:, 512 * qh:512 * (qh + 1)],
                                lhsT=kcat[:, 128 * c:128 * (c + 1)],
                                rhs=qcat[:, 512 * qh:512 * (qh + 1)],
                                start=True, stop=True)
                        p_sb = ptp.tile([128, L], f32r, tag="p_sb", name="p_sb")
                        nc.scalar.activation(p_sb[:], s_ps[:], AF.Exp, scale=SCALE)
                        for qh in range(2):
                            nc.tensor.matmul(
                                o_ps[qh][0:65, :],
                                lhsT=vext[:, 65 * c:65 * (c + 1)],
                                rhs=p_sb[:, 512 * qh:512 * (qh + 1)],
                                start=(c == 0), stop=(c == NCH - 1))

                    # copy attention accumulator out of PSUM promptly
                    oc = stp.tile([65, L], f32, tag="st", name="oc")
                    for qh in range(2):
                        nc.vector.tensor_copy(
                            oc[:, 512 * qh:512 * (qh + 1)], o_ps[qh][:])
                    # finalize the PREVIOUS head here so the single-lane ACT
                    # row ops never head-of-line-block this head's exps
                    if pend:
                        (ph, poc), = pend.items()
                        finalize(ph, poc)
                        pend.clear()
                    pend[h] = oc
                return run

            def tail():
                (ph, poc), = pend.items()
                finalize(ph, poc)
                pend.clear()
            return [head(h) for h in range(H)] + [tail]

        def oproj_blocks(m, OT, wo_t):
            def block(qc):
                def run():
                    ost = stp.tile([128, L], f32, tag="st", name="ost")
                    op_ps = [psum.tile([128, 512], f32, tag="fine",
                                       name="pp", bufs=4) for _ in range(2)]
                    for c in range(NCH):
                        for dh in range(2):
                            nc.tensor.matmul(
                                op_ps[dh][:],
                                lhsT=OT[:, L * c + 128 * qc:L * c + 128 * (qc + 1)],
                                rhs=wo_t[:, D * c + 512 * dh:D * c + 512 * (dh + 1)],
                                start=(c == 0), stop=(c == NCH - 1))
                    for dh in range(2):
                        nc.vector.tensor_copy(
                            ost[:, 512 * dh:512 * (dh + 1)], op_ps[dh][:])
                    nc.scalar.dma_start(
                        out=out[128 * qc:128 * (qc + 1), m, :], in_=ost[:])
                return run
            return [block(qc) for qc in range(NCH)]

        for _rep in range(repeat):
            # ---------------- emission schedule ----------------
            OTs = {}

            def mk_ot(m):
                OTs[m] = xts.tile([128, NCH * L], f32r, tag="xts", name="OT")

            # P0 stream 1, then weave remaining P0 streams with the first
            # projection groups.  All three xT streams stay SBUF-resident
            # (3 shared slots with the OT tiles).  Each weight loads once.
            for b in p0_blocks(1):
                b()
            wq_t = load_w(Wq)
            _interleave(proj_qk_blocks(wq_t, bq_t, 1, qT_d, xt_tiles[1]),
                        p0_blocks(2))
            _interleave(proj_qk_blocks(wq_t, bq_t, 2, qT_d, xt_tiles[2]),
                        p0_blocks(0))
            for b in proj_qk_blocks(wq_t, bq_t, 0, qT_d, xt_tiles[0]):
                b()
            wk_t = load_w(Wk)
            for s in (1, 2, 0):
                for b in proj_qk_blocks(wk_t, bk_t, s, kT_d, xt_tiles[s]):
                    b()
            wv_t = load_w(Wv)
            for b in proj_v_blocks(wv_t, 0, xt_tiles[0]):
                b()

            # A0 || (v1, v2): xt1/xt2 and Wv still resident
            mk_ot(0)

            def chain_emit():
                blocks = []
                for lc in range(NCH):
                    blocks.append(lambda lc=lc: proj_v_blocks(
                        wv_t, 1, xt_tiles[1])[lc]())
                for lc in range(NCH):
                    blocks.append(lambda lc=lc: proj_v_blocks(
                        wv_t, 2, xt_tiles[2])[lc]())
                return blocks

            _interleave(attention_blocks(0, OTs[0]), chain_emit())

            # A1 || (load Wo, oproj 0)
            mk_ot(1)
            wo_state = {}

            def o0_blocks():
                blocks = []

                def loadwo():
                    wo_state["w"] = load_w(Wo)
                blocks.append(loadwo)
                for qc in range(NCH):
                    blocks.append(lambda qc=qc: oproj_blocks(
                        0, OTs[0], wo_state["w"])[qc]())
                return blocks

            _interleave(attention_blocks(1, OTs[1]), o0_blocks())

            # A2 || oproj 1
            mk_ot(2)
            _interleave(
                attention_blocks(2, OTs[2]),
                [lambda qc=qc: oproj_blocks(1, OTs[1], wo_state["w"])[qc]()
                 for qc in range(NCH)])

            for qc in range(NCH):
                oproj_blocks(2, OTs[2], wo_state["w"])[qc]()

        for p in reversed(cstack):
            p.release()

    _split_excess_waits(nc, max_waits=1)
    return nc


def get_program():
    if "nc" not in _CACHE:
        _CACHE["nc"] = _build_program()
    return _CACHE["nc"]


def kernel(x, Wq, bq, Wk, bk, Wv, bv, Wo, bo):
    from concourse.bass_utils import run_bass_kernel_spmd

    nc = get_program()
    x = np.ascontiguousarray(np.asarray(x, dtype=np.float32))
    ws = {n: np.ascontiguousarray(np.asarray(a, dtype=np.float32))
          for n, a in (("Wq", Wq), ("Wk", Wk), ("Wv", Wv), ("Wo", Wo),
                       ("bq", bq), ("bk", bk), ("bv", bv), ("bo", bo))}
    in_maps = [dict(ws, x=np.ascontiguousarray(x[b])) for b in range(N_CORES)]
    res = run_bass_kernel_spmd(nc, in_maps, list(range(N_CORES)))
    outp = np.stack([res.results[b]["out"] for b in range(N_CORES)], axis=0)
    # bv and bo fold into a constant output row: softmax rows sum to 1, so
    # attention(v + bv) = attention(v) + bv, and (o + bv) @ Wo + bo adds
    # (bv @ Wo + bo) to every output row.
    corr = ws["bv"].astype(np.float64) @ ws["Wo"].astype(np.float64) \
        + ws["bo"].astype(np.float64)
    if np.any(corr):
        outp = (outp.astype(np.float64) + corr[None, None, None, :]).astype(
            np.float32)
    return outp



# revision 8
# speedup vs baseline: 1.0038x; 1.0038x over previous
"""Trainium2 Bass kernel for nn_MultiHeadedAttention_41566693491186.

Three dual-score MHAs over the streams packed in x[:, :, 0:3, :], with shared
Wq/Wk/Wv/Wo. Data-parallel over batch B=8: one batch element per NeuronCore.

v2 changes vs baseline:
  - x is transposed on the host and fed as xT[3, D, L]; the on-device PE
    transpose phase (P0) is gone.
  - softmax denominators: recip = reciprocal_approx_fast on DVE (1 inst)
    instead of ACT Ln+Exp; broadcast matmul runs f32r (1 cy/row) instead of
    f32 (4 cy/row).

Per-core plan (all matmuls float32r, ~tf32 precision, 1 cyc/row at N>=512):
  P1  projections (interleaved with attention below):
        qT[s] = (x_s @ Wq)^T, kT[s] = (x_s @ Wk)^T   (W-stationary, out [j, L])
        v[s]  =  x_s @ Wv                            (x-stationary, out [L, j])
      spilled to DRAM; v in an interleaved [64 data | 1 ones] per-head layout
      so the PV matmul's lhsT picks up a ones column that produces the softmax
      denominators as PSUM row 64 for free.
  P2  per (mha, head): S^T = kcat^T-chunks x qcat -> exp (ACT, scale=1/16)
      -> P^T in SBUF -> PV accumulate o^T[d, q] + sums row.  Softmax denom:
      recip row on DVE, broadcast to 64 partitions via K=1 f32r outer product,
      normalize with one DVE mul into OT.
  P3  out = OT^T @ Wo + bo  (OT-stationary, out [q, d_model]) -> DRAM.
"""

import sys

if "/opt/trn_rl_repo" not in sys.path:
    sys.path.insert(0, "/opt/trn_rl_repo")

import numpy as np

B, L, D = 8, 1024, 1024
H, DH = 16, 64
NCH = 8            # 128-sized chunks along D or L
SCALE = 0.0625     # (1/sqrt(64)) * 0.5
N_CORES = 8
# mha m reads (A, B, V) streams: q1/k1 from A, q2/k2 from B, v from V
MHA_STREAMS = ((1, 2, 0), (0, 2, 1), (0, 1, 2))
import os
FINALIZE = os.environ.get("FINALIZE", "recip_dve")

_CACHE = {}


def _split_excess_waits(nc, max_waits=1):
    """Stock neuronxcc walrus rejects instructions carrying more than
    `max_waits` semaphore waits; move excess onto same-engine NOPs."""
    import concourse.mybir as mybir

    for f in nc.m.functions:
        for bb in f.blocks:
            out = []
            changed = False
            for inst in bb.instructions:
                si = inst.sync_info
                waits = list(si.on_wait) if (si is not None and si.on_wait) else []
                if len(waits) > max_waits:
                    extra, keep = waits[:-max_waits], waits[-max_waits:]
                    k = 0
                    while extra:
                        chunk, extra = extra[:max_waits], extra[max_waits:]
                        nop = mybir.InstNoOp(
                            name=f"{inst.name}-ws{k}",
                            engine=inst.engine,
                            sync_info=mybir.SyncInfo(on_wait=chunk, on_update=[]),
                        )
                        out.append(nop)
                        k += 1
                    inst.sync_info = mybir.SyncInfo(
                        on_wait=keep,
                        on_update=list(si.on_update) if si.on_update else [],
                    )
                    changed = True
                out.append(inst)
            if changed:
                bb.instructions = out


def _interleave(*seqs):
    """Proportional merge of thunk lists, preserving within-list order."""
    items = []
    for si, seq in enumerate(seqs):
        n = len(seq)
        for i, thunk in enumerate(seq):
            items.append(((i + 0.5) / n, si, i, thunk))
    for _, _, _, t in sorted(items, key=lambda z: (z[0], z[1], z[2])):
        t()


def _build_program(repeat=1):
    import concourse.bass as bass
    import concourse.mybir as mybir
    import concourse.tile as tile

    f32 = mybir.dt.float32
    f32r = mybir.dt.float32r
    AF = mybir.ActivationFunctionType

    nc = bass.Bass("TRN2", target_bir_lowering=False, debug=False)

    xT = nc.declare_dram_parameter("xT", [3, D, L], f32r, isOutput=False)
    Wq = nc.declare_dram_parameter("Wq", [D, D], f32r, isOutput=False)
    Wk = nc.declare_dram_parameter("Wk", [D, D], f32r, isOutput=False)
    Wv = nc.declare_dram_parameter("Wv", [D, D], f32r, isOutput=False)
    Wo = nc.declare_dram_parameter("Wo", [D, D], f32r, isOutput=False)
    bq = nc.declare_dram_parameter("bq", [D], f32, isOutput=False)
    bk = nc.declare_dram_parameter("bk", [D], f32, isOutput=False)
    out = nc.declare_dram_parameter("out", [L, 3, D], f32, isOutput=True)

    # internal DRAM spill
    qT_d = [nc.dram_tensor(f"qT{s}", [D, L], f32r) for s in range(3)]
    kT_d = [nc.dram_tensor(f"kT{s}", [D, L], f32r) for s in range(3)]
    # v: head h data at cols 65h..65h+64, ones column at 65h+64
    v_d = [nc.dram_tensor(f"v{s}", [L, H * 65], f32r) for s in range(3)]

    with tile.TileContext(nc) as tc:
        cstack = []
        cp = tc.alloc_tile_pool(name="const", bufs=1)
        psum = tc.alloc_tile_pool(name="psum", bufs=1, space="PSUM")
        xts = tc.alloc_tile_pool(name="xts", bufs=3)
        cstack += [cp, psum, xts]

        cmisc = cp.tile([128, 80], f32, tag="cmisc", name="cmisc")
        ones64 = cmisc[:, 0:64]
        ones16 = cmisc[:, 0:16]
        bq_t = cmisc[:, 64:72]
        bk_t = cmisc[:, 72:80]
        nc.gpsimd.memset(ones64, 1.0)
        nc.sync.dma_start(out=bq_t, in_=bq.rearrange("(c p) -> p c", p=128))
        nc.sync.dma_start(out=bk_t, in_=bk.rearrange("(c p) -> p c", p=128))


        # ---------------- xT loads (DMA only; host pre-transposed) ----------
        xt_tiles = {}

        def load_xt(s):
            xt = xts.tile([128, NCH * L], f32r, tag="xts", name=f"xt{s}")
            d3 = xt[:].rearrange("p (c l) -> p c l", l=L)
            s3 = xT[s].rearrange("(c p) l -> p c l", p=128)
            nc.sync.dma_start(out=d3[:, 0:4, :], in_=s3[:, 0:4, :])
            nc.scalar.dma_start(out=d3[:, 4:8, :], in_=s3[:, 4:8, :])
            xt_tiles[s] = xt

        # ---------------- shared pools for P1/P2/P3 ----------------
        wrp = tc.alloc_tile_pool(name="wrp", bufs=1)
        stp = tc.alloc_tile_pool(name="stp", bufs=5)
        qkp = tc.alloc_tile_pool(name="qkp", bufs=3)
        ptp = tc.alloc_tile_pool(name="ptp", bufs=3)
        rbp = tc.alloc_tile_pool(name="rbp", bufs=2)
        cstack += [wrp, stp, qkp, ptp, rbp]

        def load_w(Wsrc):
            w_t = wrp.tile([128, NCH * D], f32r, tag="W", name="W")
            d3 = w_t[:].rearrange("p (c d) -> p c d", d=D)
            s3 = Wsrc.rearrange("(c p) d -> p c d", p=128)
            nc.sync.dma_start(out=d3[:, 0:4, :], in_=s3[:, 0:4, :])
            nc.scalar.dma_start(out=d3[:, 4:8, :], in_=s3[:, 4:8, :])
            return w_t

        def proj_qk_blocks(w_t, b_t, s, outd, xt):
            # out [j, L] = (x_s @ W)^T, one block per jc
            def block(jc):
                def run():
                    st = stp.tile([128, L], f32r, tag="st", name="st")
                    ps = [psum.tile([128, 512], f32, tag="fine",
                                    name="pp", bufs=4) for _ in range(2)]
                    for c in range(NCH):
                        for lh in range(2):
                            nc.tensor.matmul(
                                ps[lh][:],
                                lhsT=w_t[:, D * c + 128 * jc:D * c + 128 * (jc + 1)],
                                rhs=xt[:, L * c + 512 * lh:L * c + 512 * (lh + 1)],
                                start=(c == 0), stop=(c == NCH - 1))
                    for lh in range(2):
                        nc.vector.tensor_scalar_add(
                            st[:, 512 * lh:512 * (lh + 1)], ps[lh][:],
                            b_t[:, jc:jc + 1])
                    nc.scalar.dma_start(
                        out=outd[s][128 * jc:128 * (jc + 1), :], in_=st[:])
                return run
            return [block(jc) for jc in range(NCH)]

        def proj_v_blocks(w_t, s, xt):
            # out [L, j] with interleaved ones, one block per lc
            def block(lc):
                def run():
                    ps = [psum.tile([128, 512], f32, tag="fine",
                                    name="pp", bufs=4) for _ in range(2)]
                    for c in range(NCH):
                        for jh in range(2):
                            nc.tensor.matmul(
                                ps[jh][:],
                                lhsT=xt[:, L * c + 128 * lc:L * c + 128 * (lc + 1)],
                                rhs=w_t[:, D * c + 512 * jh:D * c + 512 * (jh + 1)],
                                start=(c == 0), stop=(c == NCH - 1))
                    for jh in range(2):
                        vst = stp.tile([128, 8 * 65], f32r, tag="st", name="vst")
                        r = vst[:].rearrange("p (h w) -> p h w", w=65)
                        q3 = ps[jh][:].rearrange(
                            "p (h w) -> p h w", w=64)
                        nc.vector.tensor_copy(r[:, :, 0:64], q3)
                        nc.vector.tensor_copy(
                            r[:, :, 64:65].squeeze(2), ones16[:, 0:8])
                        nc.scalar.dma_start(
                            out=v_d[s][128 * lc:128 * (lc + 1),
                                       8 * 65 * jh:8 * 65 * (jh + 1)],
                            in_=vst[:])
                return run
            return [block(lc) for lc in range(NCH)]

        def attention_blocks(m, OT):
            sa, sb, sv = MHA_STREAMS[m]
            pend = {}

            def finalize(h, oc):
                # softmax denominators: recip row on DVE; broadcast to 64
                # partitions on GPSIMD; one DVE mul normalizes into OT
                po, co = 64 * (h % 2), (h // 2) * L
                if FINALIZE == "recip_dve":
                    rb = rbp.tile([1, L], f32, tag="rb", name="rb")
                    rbb = rbp.tile([64, L], f32, tag="rbb", name="rbb")
                    nc.vector.reciprocal_approx_fast(rb[0:1, :], oc[64:65, :])
                    nc.gpsimd.partition_broadcast(rbb[:], rb[0:1, :])
                    for qh in range(2):
                        nc.vector.tensor_mul(
                            OT[po:po + 64, co + 512 * qh:co + 512 * (qh + 1)],
                            oc[0:64, 512 * qh:512 * (qh + 1)],
                            rbb[:, 512 * qh:512 * (qh + 1)])
                    return
                # baseline ACT path: recip = exp(-ln(sums)), broadcast via
                # K=1 f32 outer product
                rb = rbp.tile([64, L], f32, tag="rbb", name="rb")
                nc.scalar.activation(rb[32:33, :], oc[64:65, :], AF.Ln)
                nc.scalar.activation(rb[0:1, :], rb[32:33, :],
                                     AF.Exp, scale=-1.0)
                for qh in range(2):
                    rb_ps = psum.tile([64, 512], f32, tag="fine",
                                      name="rb_ps", bufs=4)
                    nc.tensor.matmul(
                        rb_ps[:], lhsT=ones64[0:1, :],
                        rhs=rb[0:1, 512 * qh:512 * (qh + 1)],
                        start=True, stop=True)
                    nc.vector.tensor_mul(
                        OT[po:po + 64, co + 512 * qh:co + 512 * (qh + 1)],
                        oc[0:64, 512 * qh:512 * (qh + 1)], rb_ps[:])

            def head(h):
                def run():
                    qcat = qkp.tile([128, L], f32r, tag="qcat", name="qcat")
                    kcat = qkp.tile([128, L], f32r, tag="kcat", name="kcat")
                    nc.sync.dma_start(
                        out=qcat[0:64, :], in_=qT_d[sa][64 * h:64 * h + 64, :])
                    nc.sync.dma_start(
                        out=qcat[64:128, :], in_=qT_d[sb][64 * h:64 * h + 64, :])
                    nc.sync.dma_start(
                        out=kcat[0:64, :], in_=kT_d[sa][64 * h:64 * h + 64, :])
                    nc.sync.dma_start(
                        out=kcat[64:128, :], in_=kT_d[sb][64 * h:64 * h + 64, :])
                    vext = qkp.tile([128, NCH * 65], f32r, tag="vext", name="vext", bufs=2)
                    vsrc = v_d[sv].rearrange("(c p) w -> p c w", p=128)
                    nc.sync.dma_start(
                        out=vext[:].rearrange("p (c w) -> p c w", w=65),
                        in_=vsrc[:, :, 65 * h:65 * (h + 1)])

                    o_ps = [psum.tile([65, 512], f32, tag="fine",
                                      name="ops", bufs=4) for _ in range(2)]
                    for c in range(NCH):
                        s_ps = psum.tile([128, L], f32, tag="scr", name="scr", bufs=2)
                        for qh in range(2):
                            nc.tensor.matmul(
                                s_ps[:, 512 * qh:512 * (qh + 1)],
                                lhsT=kcat[:, 128 * c:128 * (c + 1)],
                                rhs=qcat[:, 512 * qh:512 * (qh + 1)],
                                start=True, stop=True)
                        p_sb = ptp.tile([128, L], f32r, tag="p_sb", name="p_sb")
                        nc.scalar.activation(p_sb[:], s_ps[:], AF.Exp, scale=SCALE)
                        for qh in range(2):
                            nc.tensor.matmul(
                                o_ps[qh][0:65, :],
                                lhsT=vext[:, 65 * c:65 * (c + 1)],
                                rhs=p_sb[:, 512 * qh:512 * (qh + 1)],
                                start=(c == 0), stop=(c == NCH - 1))

                    # copy attention accumulator out of PSUM promptly
                    oc = stp.tile([65, L], f32, tag="st", name="oc")
                    for qh in range(2):
                        nc.vector.tensor_copy(
                            oc[:, 512 * qh:512 * (qh + 1)], o_ps[qh][:])
                    # finalize the PREVIOUS head here so the single-lane DVE
                    # row ops never head-of-line-block this head's copies
                    if pend:
                        (ph, poc), = pend.items()
                        finalize(ph, poc)
                        pend.clear()
                    pend[h] = oc
                return run

            def tail():
                (ph, poc), = pend.items()
                finalize(ph, poc)
                pend.clear()
            return [head(h) for h in range(H)] + [tail]

        def oproj_blocks(m, OT, wo_t):
            def block(qc):
                def run():
                    ost = stp.tile([128, L], f32, tag="st", name="ost")
                    op_ps = [psum.tile([128, 512], f32, tag="fine",
                                       name="pp", bufs=4) for _ in range(2)]
                    for c in range(NCH):
                        for dh in range(2):
                            nc.tensor.matmul(
                                op_ps[dh][:],
                                lhsT=OT[:, L * c + 128 * qc:L * c + 128 * (qc + 1)],
                                rhs=wo_t[:, D * c + 512 * dh:D * c + 512 * (dh + 1)],
                                start=(c == 0), stop=(c == NCH - 1))
                    for dh in range(2):
                        nc.vector.tensor_copy(
                            ost[:, 512 * dh:512 * (dh + 1)], op_ps[dh][:])
                    nc.scalar.dma_start(
                        out=out[128 * qc:128 * (qc + 1), m, :], in_=ost[:])
                return run
            return [block(qc) for qc in range(NCH)]

        for _rep in range(repeat):
            # ---------------- emission schedule ----------------
            OTs = {}

            def mk_ot(m):
                OTs[m] = xts.tile([128, NCH * L], f32r, tag="xts", name="OT")

            load_xt(1)
            wq_t = load_w(Wq)
            load_xt(2)
            load_xt(0)
            for s in (1, 2, 0):
                for b in proj_qk_blocks(wq_t, bq_t, s, qT_d, xt_tiles[s]):
                    b()
            wk_t = load_w(Wk)
            for s in (1, 2, 0):
                for b in proj_qk_blocks(wk_t, bk_t, s, kT_d, xt_tiles[s]):
                    b()
            wv_t = load_w(Wv)
            for b in proj_v_blocks(wv_t, 0, xt_tiles[0]):
                b()

            # A0 || (v1, v2): xt1/xt2 and Wv still resident
            mk_ot(0)

            def chain_emit():
                blocks = []
                for lc in range(NCH):
                    blocks.append(lambda lc=lc: proj_v_blocks(
                        wv_t, 1, xt_tiles[1])[lc]())
                for lc in range(NCH):
                    blocks.append(lambda lc=lc: proj_v_blocks(
                        wv_t, 2, xt_tiles[2])[lc]())
                return blocks

            _interleave(attention_blocks(0, OTs[0]), chain_emit())

            # A1 || (load Wo, oproj 0)
            mk_ot(1)
            wo_state = {}

            def o0_blocks():
                blocks = []

                def loadwo():
                    wo_state["w"] = load_w(Wo)
                blocks.append(loadwo)
                for qc in range(NCH):
                    blocks.append(lambda qc=qc: oproj_blocks(
                        0, OTs[0], wo_state["w"])[qc]())
                return blocks

            _interleave(attention_blocks(1, OTs[1]), o0_blocks())

            # A2 || oproj 1
            mk_ot(2)
            _interleave(
                attention_blocks(2, OTs[2]),
                [lambda qc=qc: oproj_blocks(1, OTs[1], wo_state["w"])[qc]()
                 for qc in range(NCH)])

            for qc in range(NCH):
                oproj_blocks(2, OTs[2], wo_state["w"])[qc]()

        for p in reversed(cstack):
            p.release()

    _split_excess_waits(nc, max_waits=1)
    return nc


def get_program():
    if "nc" not in _CACHE:
        _CACHE["nc"] = _build_program()
    return _CACHE["nc"]


def kernel(x, Wq, bq, Wk, bk, Wv, bv, Wo, bo):
    from concourse.bass_utils import run_bass_kernel_spmd

    nc = get_program()
    x = np.ascontiguousarray(np.asarray(x, dtype=np.float32))
    ws = {n: np.ascontiguousarray(np.asarray(a, dtype=np.float32))
          for n, a in (("Wq", Wq), ("Wk", Wk), ("Wv", Wv), ("Wo", Wo),
                       ("bq", bq), ("bk", bk))}
    bv = np.asarray(bv, dtype=np.float64)
    bo = np.asarray(bo, dtype=np.float64)
    in_maps = [
        dict(ws, xT=np.ascontiguousarray(x[b].transpose(1, 2, 0)))
        for b in range(N_CORES)
    ]
    res = run_bass_kernel_spmd(nc, in_maps, list(range(N_CORES)))
    outp = np.stack([res.results[b]["out"] for b in range(N_CORES)], axis=0)
    # bv and bo fold into a constant output row: softmax rows sum to 1, so
    # attention(v + bv) = attention(v) + bv, and (o + bv) @ Wo + bo adds
    # (bv @ Wo + bo) to every output row.
    corr = bv @ ws["Wo"].astype(np.float64) + bo
    if np.any(corr):
        outp = (outp.astype(np.float64) + corr[None, None, None, :]).astype(
            np.float32)
    return outp


# revision 13
# speedup vs baseline: 1.1146x; 1.1103x over previous
"""Trainium2 Bass kernel for nn_MultiHeadedAttention_41566693491186.

Three dual-score MHAs over the streams packed in x[:, :, 0:3, :], with shared
Wq/Wk/Wv/Wo. Data-parallel over batch B=8: one batch element per NeuronCore.

v2 changes vs baseline:
  - x is transposed on the host and fed as xT[3, D, L]; the on-device PE
    transpose phase (P0) is gone.
  - softmax denominators: recip = reciprocal_approx_fast on DVE (1 inst)
    instead of ACT Ln+Exp; broadcast matmul runs f32r (1 cy/row) instead of
    f32 (4 cy/row).

Per-core plan (all matmuls float32r, ~tf32 precision, 1 cyc/row at N>=512):
  P1  projections (interleaved with attention below):
        qT[s] = (x_s @ Wq)^T, kT[s] = (x_s @ Wk)^T   (W-stationary, out [j, L])
        v[s]  =  x_s @ Wv                            (x-stationary, out [L, j])
      spilled to DRAM; v in an interleaved [64 data | 1 ones] per-head layout
      so the PV matmul's lhsT picks up a ones column that produces the softmax
      denominators as PSUM row 64 for free.
  P2  per (mha, head): S^T = kcat^T-chunks x qcat -> exp (ACT, scale=1/16)
      -> P^T in SBUF -> PV accumulate o^T[d, q] + sums row.  Softmax denom:
      recip row on DVE, broadcast to 64 partitions via K=1 f32r outer product,
      normalize with one DVE mul into OT.
  P3  out = OT^T @ Wo + bo  (OT-stationary, out [q, d_model]) -> DRAM.
"""

import sys

if "/opt/trn_rl_repo" not in sys.path:
    sys.path.insert(0, "/opt/trn_rl_repo")

import numpy as np

B, L, D = 8, 1024, 1024
H, DH = 16, 64
NCH = 8            # 128-sized chunks along D or L
SCALE = 0.0625     # (1/sqrt(64)) * 0.5
N_CORES = 8
# mha m reads (A, B, V) streams: q1/k1 from A, q2/k2 from B, v from V
MHA_STREAMS = ((1, 2, 0), (0, 2, 1), (0, 1, 2))
import os
FINALIZE = os.environ.get("FINALIZE", "recip_dve")

_CACHE = {}


def _split_excess_waits(nc, max_waits=1):
    """Stock neuronxcc walrus rejects instructions carrying more than
    `max_waits` semaphore waits; move excess onto same-engine NOPs."""
    import concourse.mybir as mybir

    for f in nc.m.functions:
        for bb in f.blocks:
            out = []
            changed = False
            for inst in bb.instructions:
                si = inst.sync_info
                waits = list(si.on_wait) if (si is not None and si.on_wait) else []
                if len(waits) > max_waits:
                    extra, keep = waits[:-max_waits], waits[-max_waits:]
                    k = 0
                    while extra:
                        chunk, extra = extra[:max_waits], extra[max_waits:]
                        nop = mybir.InstNoOp(
                            name=f"{inst.name}-ws{k}",
                            engine=inst.engine,
                            sync_info=mybir.SyncInfo(on_wait=chunk, on_update=[]),
                        )
                        out.append(nop)
                        k += 1
                    inst.sync_info = mybir.SyncInfo(
                        on_wait=keep,
                        on_update=list(si.on_update) if si.on_update else [],
                    )
                    changed = True
                out.append(inst)
            if changed:
                bb.instructions = out


def _interleave(*seqs):
    """Proportional merge of thunk lists, preserving within-list order."""
    items = []
    for si, seq in enumerate(seqs):
        n = len(seq)
        for i, thunk in enumerate(seq):
            items.append(((i + 0.5) / n, si, i, thunk))
    for _, _, _, t in sorted(items, key=lambda z: (z[0], z[1], z[2])):
        t()


def _build_program(repeat=1):
    import concourse.bass as bass
    import concourse.mybir as mybir
    import concourse.tile as tile

    f32 = mybir.dt.float32
    f32r = mybir.dt.float32r
    AF = mybir.ActivationFunctionType

    nc = bass.Bass("TRN2", target_bir_lowering=False, debug=False)

    xT = nc.declare_dram_parameter("xT", [3, D, L], f32r, isOutput=False)
    Wq = nc.declare_dram_parameter("Wq", [D, D], f32r, isOutput=False)
    Wk = nc.declare_dram_parameter("Wk", [D, D], f32r, isOutput=False)
    Wv = nc.declare_dram_parameter("Wv", [D, D], f32r, isOutput=False)
    Wo = nc.declare_dram_parameter("Wo", [D, D], f32r, isOutput=False)
    bq = nc.declare_dram_parameter("bq", [D], f32, isOutput=False)
    bk = nc.declare_dram_parameter("bk", [D], f32, isOutput=False)
    out = nc.declare_dram_parameter("out", [L, 3, D], f32, isOutput=True)

    # internal DRAM spill
    qT_d = [nc.dram_tensor(f"qT{s}", [D, L], f32r) for s in range(3)]
    kT_d = [nc.dram_tensor(f"kT{s}", [D, L], f32r) for s in range(3)]
    # v: head h data at cols 65h..65h+64, ones column at 65h+64
    v_d = [nc.dram_tensor(f"v{s}", [L, H * 65], f32r) for s in range(3)]

    with tile.TileContext(nc) as tc:
        cstack = []
        cp = tc.alloc_tile_pool(name="const", bufs=1)
        psum = tc.alloc_tile_pool(name="psum", bufs=1, space="PSUM")
        xts = tc.alloc_tile_pool(name="xts", bufs=3)
        cstack += [cp, psum, xts]

        cmisc = cp.tile([128, 80], f32, tag="cmisc", name="cmisc")
        ones64 = cmisc[:, 0:64]
        ones16 = cmisc[:, 0:16]
        bq_t = cmisc[:, 64:72]
        bk_t = cmisc[:, 72:80]
        nc.gpsimd.memset(ones64, 1.0)
        onesr = cp.tile([1, 64], f32r, tag="onesr", name="onesr")
        nc.vector.tensor_copy(onesr[:], ones64[0:1, :])
        ones_r = onesr
        nc.sync.dma_start(out=bq_t, in_=bq.rearrange("(c p) -> p c", p=128))
        nc.sync.dma_start(out=bk_t, in_=bk.rearrange("(c p) -> p c", p=128))


        # ---------------- xT loads (DMA only; host pre-transposed) ----------
        xt_tiles = {}

        def load_xt(s):
            xt = xts.tile([128, NCH * L], f32r, tag="xts", name=f"xt{s}")
            d3 = xt[:].rearrange("p (c l) -> p c l", l=L)
            s3 = xT[s].rearrange("(c p) l -> p c l", p=128)
            nc.sync.dma_start(out=d3[:, 0:4, :], in_=s3[:, 0:4, :])
            nc.scalar.dma_start(out=d3[:, 4:8, :], in_=s3[:, 4:8, :])
            xt_tiles[s] = xt

        # ---------------- shared pools for P1/P2/P3 ----------------
        wrp = tc.alloc_tile_pool(name="wrp", bufs=1)
        stp = tc.alloc_tile_pool(name="stp", bufs=5)
        qkp = tc.alloc_tile_pool(name="qkp", bufs=3)
        ptp = tc.alloc_tile_pool(name="ptp", bufs=3)
        rbp = tc.alloc_tile_pool(name="rbp", bufs=2)
        cstack += [wrp, stp, qkp, ptp, rbp]

        def load_w(Wsrc):
            w_t = wrp.tile([128, NCH * D], f32r, tag="W", name="W")
            d3 = w_t[:].rearrange("p (c d) -> p c d", d=D)
            s3 = Wsrc.rearrange("(c p) d -> p c d", p=128)
            nc.sync.dma_start(out=d3[:, 0:4, :], in_=s3[:, 0:4, :])
            nc.scalar.dma_start(out=d3[:, 4:8, :], in_=s3[:, 4:8, :])
            return w_t

        def proj_qk_blocks(w_t, b_t, s, outd, xt):
            # out [j, L] = (x_s @ W)^T, one block per jc
            def block(jc):
                def run():
                    st = stp.tile([128, L], f32r, tag="st", name="st")
                    ps = [psum.tile([128, 512], f32, tag="fine",
                                    name="pp", bufs=4) for _ in range(2)]
                    for c in range(NCH):
                        for lh in range(2):
                            nc.tensor.matmul(
                                ps[lh][:],
                                lhsT=w_t[:, D * c + 128 * jc:D * c + 128 * (jc + 1)],
                                rhs=xt[:, L * c + 512 * lh:L * c + 512 * (lh + 1)],
                                start=(c == 0), stop=(c == NCH - 1))
                    for lh in range(2):
                        nc.vector.tensor_scalar_add(
                            st[:, 512 * lh:512 * (lh + 1)], ps[lh][:],
                            b_t[:, jc:jc + 1])
                    nc.scalar.dma_start(
                        out=outd[s][128 * jc:128 * (jc + 1), :], in_=st[:])
                return run
            return [block(jc) for jc in range(NCH)]

        def proj_v_blocks(w_t, s, xt):
            # out [L, j] with interleaved ones, one block per lc
            def block(lc):
                def run():
                    ps = [psum.tile([128, 512], f32, tag="fine",
                                    name="pp", bufs=4) for _ in range(2)]
                    for c in range(NCH):
                        for jh in range(2):
                            nc.tensor.matmul(
                                ps[jh][:],
                                lhsT=xt[:, L * c + 128 * lc:L * c + 128 * (lc + 1)],
                                rhs=w_t[:, D * c + 512 * jh:D * c + 512 * (jh + 1)],
                                start=(c == 0), stop=(c == NCH - 1))
                    for jh in range(2):
                        vst = stp.tile([128, 8 * 65], f32r, tag="st", name="vst")
                        r = vst[:].rearrange("p (h w) -> p h w", w=65)
                        q3 = ps[jh][:].rearrange(
                            "p (h w) -> p h w", w=64)
                        nc.vector.tensor_copy(r[:, :, 0:64], q3)
                        nc.vector.tensor_copy(
                            r[:, :, 64:65].squeeze(2), ones16[:, 0:8])
                        nc.scalar.dma_start(
                            out=v_d[s][128 * lc:128 * (lc + 1),
                                       8 * 65 * jh:8 * 65 * (jh + 1)],
                            in_=vst[:])
                return run
            return [block(lc) for lc in range(NCH)]

        def attention_blocks(m, OT):
            sa, sb, sv = MHA_STREAMS[m]
            pend = {}

            def finalize(h, oc):
                # softmax denominators: recip row on DVE; broadcast to 64
                # partitions on GPSIMD; one DVE mul normalizes into OT
                po, co = 64 * (h % 2), (h // 2) * L
                rb = rbp.tile([1, L], f32, tag="rb", name="rb")
                rb_r = rbp.tile([1, L], f32r, tag="rbr", name="rbr")
                if FINALIZE == "recip_dve":
                    # exact reciprocal on DVE, rounded to f32r for the 1
                    # cyc/row broadcast matmul
                    nc.vector.reciprocal(rb[0:1, :], oc[64:65, :])
                    nc.vector.tensor_copy(rb_r[0:1, :], rb[0:1, :])
                else:
                    nc.scalar.activation(rb[0:1, :], oc[64:65, :], AF.Ln)
                    nc.scalar.activation(rb_r[0:1, :], rb[0:1, :],
                                         AF.Exp, scale=-1.0)
                for qh in range(2):
                    rb_ps = psum.tile([64, 512], f32, tag="fine",
                                      name="rb_ps", bufs=4)
                    nc.tensor.matmul(
                        rb_ps[:], lhsT=ones_r[0:1, :],
                        rhs=rb_r[0:1, 512 * qh:512 * (qh + 1)],
                        start=True, stop=True)
                    nc.vector.tensor_mul(
                        OT[po:po + 64, co + 512 * qh:co + 512 * (qh + 1)],
                        oc[0:64, 512 * qh:512 * (qh + 1)], rb_ps[:])

            def head(h):
                def run():
                    qcat = qkp.tile([128, L], f32r, tag="qcat", name="qcat")
                    kcat = qkp.tile([128, L], f32r, tag="kcat", name="kcat")
                    nc.sync.dma_start(
                        out=qcat[0:64, :], in_=qT_d[sa][64 * h:64 * h + 64, :])
                    nc.sync.dma_start(
                        out=qcat[64:128, :], in_=qT_d[sb][64 * h:64 * h + 64, :])
                    nc.sync.dma_start(
                        out=kcat[0:64, :], in_=kT_d[sa][64 * h:64 * h + 64, :])
                    nc.sync.dma_start(
                        out=kcat[64:128, :], in_=kT_d[sb][64 * h:64 * h + 64, :])
                    vext = qkp.tile([128, NCH * 65], f32r, tag="vext", name="vext", bufs=2)
                    vsrc = v_d[sv].rearrange("(c p) w -> p c w", p=128)
                    nc.sync.dma_start(
                        out=vext[:].rearrange("p (c w) -> p c w", w=65),
                        in_=vsrc[:, :, 65 * h:65 * (h + 1)])

                    o_ps = [psum.tile([65, 512], f32, tag="fine",
                                      name="ops", bufs=4) for _ in range(2)]
                    for c in range(NCH):
                        s_ps = psum.tile([128, L], f32, tag="scr", name="scr", bufs=2)
                        for qh in range(2):
                            nc.tensor.matmul(
                                s_ps[:, 512 * qh:512 * (qh + 1)],
                                lhsT=kcat[:, 128 * c:128 * (c + 1)],
                                rhs=qcat[:, 512 * qh:512 * (qh + 1)],
                                start=True, stop=True)
                        p_sb = ptp.tile([128, L], f32r, tag="p_sb", name="p_sb")
                        nc.scalar.activation(p_sb[:], s_ps[:], AF.Exp, scale=SCALE)
                        for qh in range(2):
                            nc.tensor.matmul(
                                o_ps[qh][0:65, :],
                                lhsT=vext[:, 65 * c:65 * (c + 1)],
                                rhs=p_sb[:, 512 * qh:512 * (qh + 1)],
                                start=(c == 0), stop=(c == NCH - 1))

                    # copy attention accumulator out of PSUM promptly
                    oc = stp.tile([65, L], f32, tag="st", name="oc")
                    for qh in range(2):
                        nc.vector.tensor_copy(
                            oc[:, 512 * qh:512 * (qh + 1)], o_ps[qh][:])
                    # finalize the PREVIOUS head here so the single-lane DVE
                    # row ops never head-of-line-block this head's copies
                    if pend:
                        (ph, poc), = pend.items()
                        finalize(ph, poc)
                        pend.clear()
                    pend[h] = oc
                return run

            def tail():
                (ph, poc), = pend.items()
                finalize(ph, poc)
                pend.clear()
            return [head(h) for h in range(H)] + [tail]

        def oproj_blocks(m, OT, wo_t):
            def block(qc):
                def run():
                    ost = stp.tile([128, L], f32, tag="st", name="ost")
                    op_ps = [psum.tile([128, 512], f32, tag="fine",
                                       name="pp", bufs=4) for _ in range(2)]
                    for c in range(NCH):
                        for dh in range(2):
                            nc.tensor.matmul(
                                op_ps[dh][:],
                                lhsT=OT[:, L * c + 128 * qc:L * c + 128 * (qc + 1)],
                                rhs=wo_t[:, D * c + 512 * dh:D * c + 512 * (dh + 1)],
                                start=(c == 0), stop=(c == NCH - 1))
                    for dh in range(2):
                        nc.vector.tensor_copy(
                            ost[:, 512 * dh:512 * (dh + 1)], op_ps[dh][:])
                    nc.scalar.dma_start(
                        out=out[128 * qc:128 * (qc + 1), m, :], in_=ost[:])
                return run
            return [block(qc) for qc in range(NCH)]

        for _rep in range(repeat):
            # ---------------- emission schedule ----------------
            OTs = {}

            def mk_ot(m):
                OTs[m] = xts.tile([128, NCH * L], f32r, tag="xts", name="OT")

            load_xt(1)
            wq_t = load_w(Wq)
            load_xt(2)
            load_xt(0)
            for s in (1, 2, 0):
                for b in proj_qk_blocks(wq_t, bq_t, s, qT_d, xt_tiles[s]):
                    b()
            wk_t = load_w(Wk)
            for s in (1, 2, 0):
                for b in proj_qk_blocks(wk_t, bk_t, s, kT_d, xt_tiles[s]):
                    b()
            wv_t = load_w(Wv)
            for b in proj_v_blocks(wv_t, 0, xt_tiles[0]):
                b()

            # A0 || (v1, v2): xt1/xt2 and Wv still resident
            mk_ot(0)

            def chain_emit():
                blocks = []
                for lc in range(NCH):
                    blocks.append(lambda lc=lc: proj_v_blocks(
                        wv_t, 1, xt_tiles[1])[lc]())
                for lc in range(NCH):
                    blocks.append(lambda lc=lc: proj_v_blocks(
                        wv_t, 2, xt_tiles[2])[lc]())
                return blocks

            _interleave(attention_blocks(0, OTs[0]), chain_emit())

            # A1 || (load Wo, oproj 0)
            mk_ot(1)
            wo_state = {}

            def o0_blocks():
                blocks = []

                def loadwo():
                    wo_state["w"] = load_w(Wo)
                blocks.append(loadwo)
                for qc in range(NCH):
                    blocks.append(lambda qc=qc: oproj_blocks(
                        0, OTs[0], wo_state["w"])[qc]())
                return blocks

            _interleave(attention_blocks(1, OTs[1]), o0_blocks())

            # A2 || oproj 1
            mk_ot(2)
            _interleave(
                attention_blocks(2, OTs[2]),
                [lambda qc=qc: oproj_blocks(1, OTs[1], wo_state["w"])[qc]()
                 for qc in range(NCH)])

            for qc in range(NCH):
                oproj_blocks(2, OTs[2], wo_state["w"])[qc]()

        for p in reversed(cstack):
            p.release()

    _split_excess_waits(nc, max_waits=1)
    return nc


def get_program():
    if "nc" not in _CACHE:
        _CACHE["nc"] = _build_program()
    return _CACHE["nc"]


def kernel(x, Wq, bq, Wk, bk, Wv, bv, Wo, bo):
    from concourse.bass_utils import run_bass_kernel_spmd

    nc = get_program()
    x = np.ascontiguousarray(np.asarray(x, dtype=np.float32))
    ws = {n: np.ascontiguousarray(np.asarray(a, dtype=np.float32))
          for n, a in (("Wq", Wq), ("Wk", Wk), ("Wv", Wv), ("Wo", Wo),
                       ("bq", bq), ("bk", bk))}
    bv = np.asarray(bv, dtype=np.float64)
    bo = np.asarray(bo, dtype=np.float64)
    in_maps = [
        dict(ws, xT=np.ascontiguousarray(x[b].transpose(1, 2, 0)))
        for b in range(N_CORES)
    ]
    res = run_bass_kernel_spmd(nc, in_maps, list(range(N_CORES)))
    outp = np.stack([res.results[b]["out"] for b in range(N_CORES)], axis=0)
    # bv and bo fold into a constant output row: softmax rows sum to 1, so
    # attention(v + bv) = attention(v) + bv, and (o + bv) @ Wo + bo adds
    # (bv @ Wo + bo) to every output row.
    corr = bv @ ws["Wo"].astype(np.float64) + bo
    if np.any(corr):
        outp = (outp.astype(np.float64) + corr[None, None, None, :]).astype(
            np.float32)
    return outp


# revision 19
# speedup vs baseline: 1.2732x; 1.1423x over previous
"""Trainium2 Bass kernel for nn_MultiHeadedAttention_41566693491186.

Three dual-score MHAs over the streams packed in x[:, :, 0:3, :], with shared
Wq/Wk/Wv/Wo. Data-parallel over batch B=8: one batch element per NeuronCore.

v3 design:
  - Host precomputes xT = x^T per stream and splits xT and 32*W{q,k,v} into
    fp8e4m3 (hi, lo) pairs: A ~= hi + lo with ~0.15% residual.  The nine
    input projections run as fp8 DoubleRow matmuls (2 k-tiles per pass)
    keeping hi*hi + hi*lo + lo*hi cross terms: 12 DR matmuls per [128,512]
    output tile vs 16 f32r matmuls, at near-bf16 accuracy.
  - The 32x weight scale cancels exactly: exp scale becomes 2^-14 (q and k
    both carry 32x), and the v ones-column is 32.0 so softmax denominators
    scale with the numerators.
  - All attention-side tensors (qT/kT/v spills, qcat/kcat/vext, p, OT) are
    bf16: same 1 cyc/row PE cost as f32r, half the DMA/SBUF.
  - Softmax denominators: exact DVE reciprocal + f32r K=1 broadcast matmul
    (1 cyc/row), normalize with one DVE mul into OT.
  - QK^T / PV / out-projection stay f32r-grade (bf16 inputs, f32 PSUM).

Per-core plan:
  P1  projections (interleaved with attention below):
        qT[s] = (32 x_s Wq)^T, kT[s] = (32 x_s Wk)^T  (W-stationary, [j, L])
        v[s]  =  32 x_s Wv    (x-stationary, out [L, j], interleaved with a
                               32.0 column per head for the denominators)
  P2  per (mha, head): S^T = kcat^T-chunks x qcat -> exp (ACT, scale 2^-14,
      bf16 out) -> PV accumulate o^T[d, q] + sums row -> DVE recip ->
      f32r broadcast -> DVE mul into OT (bf16).
  P3  out = OT^T @ Wo + bo  (OT-stationary, out [q, d_model]) -> DRAM.
"""

import sys

if "/opt/trn_rl_repo" not in sys.path:
    sys.path.insert(0, "/opt/trn_rl_repo")

import numpy as np

B, L, D = 8, 1024, 1024
H, DH = 16, 64
NCH = 8              # 128-sized chunks along D or L
SCALE = 0.0625 / 1024.0   # (1/sqrt(64)) * 0.5 / (32*32)
WSCL = 32.0
N_CORES = 8
# mha m reads (A, B, V) streams: q1/k1 from A, q2/k2 from B, v from V
MHA_STREAMS = ((1, 2, 0), (0, 2, 1), (0, 1, 2))

_CACHE = {}


def _split_excess_waits(nc, max_waits=1):
    """Stock neuronxcc walrus rejects instructions carrying more than
    `max_waits` semaphore waits; move excess onto same-engine NOPs."""
    import concourse.mybir as mybir

    for f in nc.m.functions:
        for bb in f.blocks:
            out = []
            changed = False
            for inst in bb.instructions:
                si = inst.sync_info
                waits = list(si.on_wait) if (si is not None and si.on_wait) else []
                if len(waits) > max_waits:
                    extra, keep = waits[:-max_waits], waits[-max_waits:]
                    k = 0
                    while extra:
                        chunk, extra = extra[:max_waits], extra[max_waits:]
                        nop = mybir.InstNoOp(
                            name=f"{inst.name}-ws{k}",
                            engine=inst.engine,
                            sync_info=mybir.SyncInfo(on_wait=chunk, on_update=[]),
                        )
                        out.append(nop)
                        k += 1
                    inst.sync_info = mybir.SyncInfo(
                        on_wait=keep,
                        on_update=list(si.on_update) if si.on_update else [],
                    )
                    changed = True
                out.append(inst)
            if changed:
                bb.instructions = out


def _interleave(*seqs):
    """Proportional merge of thunk lists, preserving within-list order."""
    items = []
    for si, seq in enumerate(seqs):
        n = len(seq)
        for i, thunk in enumerate(seq):
            items.append(((i + 0.5) / n, si, i, thunk))
    for _, _, _, t in sorted(items, key=lambda z: (z[0], z[1], z[2])):
        t()


def _build_program(repeat=1):
    import concourse.bass as bass
    import concourse.mybir as mybir
    import concourse.tile as tile

    f32 = mybir.dt.float32
    f32r = mybir.dt.float32r
    bf16 = mybir.dt.bfloat16
    f8 = mybir.dt.float8e4
    DR = mybir.MatmulPerfMode.DoubleRow
    AF = mybir.ActivationFunctionType

    nc = bass.Bass("TRN2", target_bir_lowering=False, debug=False)

    # hi/lo fp8 pairs, packed [2, D, L]: index 0 = hi, 1 = lo
    xT8 = nc.declare_dram_parameter("xT8", [3, 2, D, L], f8, isOutput=False)
    Wq8 = nc.declare_dram_parameter("Wq8", [2, D, D], f8, isOutput=False)
    Wk8 = nc.declare_dram_parameter("Wk8", [2, D, D], f8, isOutput=False)
    Wv8 = nc.declare_dram_parameter("Wv8", [2, D, D], f8, isOutput=False)
    Wo = nc.declare_dram_parameter("Wo", [D, D], bf16, isOutput=False)
    bq = nc.declare_dram_parameter("bq", [D], f32, isOutput=False)
    bk = nc.declare_dram_parameter("bk", [D], f32, isOutput=False)
    out = nc.declare_dram_parameter("out", [L, 3, D], f32, isOutput=True)

    # internal DRAM spill (bf16)
    qT_d = [nc.dram_tensor(f"qT{s}", [D, L], bf16) for s in range(3)]
    kT_d = [nc.dram_tensor(f"kT{s}", [D, L], bf16) for s in range(3)]
    # v: head h data at cols 65h..65h+64, 32.0 column at 65h+64
    v_d = [nc.dram_tensor(f"v{s}", [L, H * 65], bf16) for s in range(3)]

    with tile.TileContext(nc) as tc:
        cstack = []
        cp = tc.alloc_tile_pool(name="const", bufs=1)
        psum = tc.alloc_tile_pool(name="psum", bufs=1, space="PSUM")
        xts = tc.alloc_tile_pool(name="xts", bufs=4)
        cstack += [cp, psum, xts]

        cmisc = cp.tile([128, 96], f32, tag="cmisc", name="cmisc")
        ones64 = cmisc[:, 0:64]
        v32c = cmisc[:, 80:96]
        bq_t = cmisc[:, 64:72]
        bk_t = cmisc[:, 72:80]
        nc.gpsimd.memset(ones64, 1.0)
        nc.gpsimd.memset(v32c, WSCL)
        onesr = cp.tile([1, 64], f32r, tag="onesr", name="onesr")
        nc.vector.tensor_copy(onesr[:], ones64[0:1, :])
        ones_r = onesr
        nc.sync.dma_start(out=bq_t, in_=bq.rearrange("(c p) -> p c", p=128))
        nc.sync.dma_start(out=bk_t, in_=bk.rearrange("(c p) -> p c", p=128))

        # ---------------- xT loads (fp8 hi+lo, host pre-transposed) ---------
        xt_tiles = {}

        def load_xt(s):
            # [128, hilo, c, l] fp8
            xt = xts.tile([128, 2, NCH, L], f8, tag="xts", name=f"xt{s}")
            src = xT8[s].rearrange("t (c p) l -> p t c l", p=128)
            for t in range(2):
                nc.sync.dma_start(out=xt[:, t, 0:4, :], in_=src[:, t, 0:4, :])
                nc.scalar.dma_start(out=xt[:, t, 4:8, :], in_=src[:, t, 4:8, :])
            xt_tiles[s] = xt

        # ---------------- shared pools for P1/P2/P3 ----------------
        wrp = tc.alloc_tile_pool(name="wrp", bufs=1)
        stp = tc.alloc_tile_pool(name="stp", bufs=5)
        qkp = tc.alloc_tile_pool(name="qkp", bufs=3)
        ptp = tc.alloc_tile_pool(name="ptp", bufs=3)
        rbp = tc.alloc_tile_pool(name="rbp", bufs=2)
        cstack += [wrp, stp, qkp, ptp, rbp]

        def load_w8(Wsrc):
            # [128, hilo, c, d] fp8
            w_t = wrp.tile([128, 2, NCH, D], f8, tag="W8", name="W8")
            src = Wsrc.rearrange("t (c p) d -> p t c d", p=128)
            for t in range(2):
                nc.sync.dma_start(out=w_t[:, t, 0:4, :], in_=src[:, t, 0:4, :])
                nc.scalar.dma_start(out=w_t[:, t, 4:8, :], in_=src[:, t, 4:8, :])
            return w_t

        def load_wo(Wsrc):
            w_t = wrp.tile([128, NCH * D], bf16, tag="Wor", name="Wor")
            d3 = w_t[:].rearrange("p (c d) -> p c d", d=D)
            s3 = Wsrc.rearrange("(c p) d -> p c d", p=128)
            nc.sync.dma_start(out=d3[:, 0:4, :], in_=s3[:, 0:4, :])
            nc.scalar.dma_start(out=d3[:, 4:8, :], in_=s3[:, 4:8, :])
            return w_t

        def dr_products(emit, w_t, xt):
            """12 DoubleRow matmuls accumulating hi*hi + hi*lo + lo*hi over
            4 chunk-pairs; emit(lhs_sel, rhs_sel, t, first, last)."""
            combos = ((0, 0), (0, 1), (1, 0))
            n = 0
            for t in range(4):
                for (wi, xi) in combos:
                    n += 1
                    emit(wi, xi, t, n == 1, n == 12)

        def proj_qk_blocks(w_t, b_t, s, outd, xt):
            # out [j, L] = (32 x_s W)^T, one block per jc
            def block(jc):
                def run():
                    st = stp.tile([128, L], bf16, tag="stq", name="st")
                    ps = [psum.tile([128, 512], f32, tag="fine",
                                    name="pp", bufs=4) for _ in range(2)]

                    def emit(wi, xi, t, first, last):
                        for lh in range(2):
                            nc.tensor.matmul(
                                ps[lh][:],
                                lhsT=w_t[:, wi, 2 * t:2 * t + 2,
                                         128 * jc:128 * (jc + 1)],
                                rhs=xt[:, xi, 2 * t:2 * t + 2,
                                       512 * lh:512 * (lh + 1)],
                                start=first, stop=last, perf_mode=DR)
                    dr_products(emit, w_t, xt)
                    for lh in range(2):
                        nc.vector.tensor_scalar_add(
                            st[:, 512 * lh:512 * (lh + 1)], ps[lh][:],
                            b_t[:, jc:jc + 1])
                    nc.scalar.dma_start(
                        out=outd[s][128 * jc:128 * (jc + 1), :], in_=st[:])
                return run
            return [block(jc) for jc in range(NCH)]

        def proj_v_blocks(w_t, s, xt):
            # out [L, j] with interleaved 32.0 columns, one block per lc
            def block(lc):
                def run():
                    ps = [psum.tile([128, 512], f32, tag="fine",
                                    name="pp", bufs=4) for _ in range(2)]

                    def emit(wi, xi, t, first, last):
                        for jh in range(2):
                            nc.tensor.matmul(
                                ps[jh][:],
                                lhsT=xt[:, xi, 2 * t:2 * t + 2,
                                        128 * lc:128 * (lc + 1)],
                                rhs=w_t[:, wi, 2 * t:2 * t + 2,
                                        512 * jh:512 * (jh + 1)],
                                start=first, stop=last, perf_mode=DR)
                    dr_products(emit, w_t, xt)
                    for jh in range(2):
                        vst = stp.tile([128, 8 * 65], bf16, tag="stv", name="vst")
                        r = vst[:].rearrange("p (h w) -> p h w", w=65)
                        q3 = ps[jh][:].rearrange(
                            "p (h w) -> p h w", w=64)
                        nc.vector.tensor_copy(r[:, :, 0:64], q3)
                        nc.vector.tensor_copy(
                            r[:, :, 64:65].squeeze(2), v32c[:, 0:8])
                        nc.scalar.dma_start(
                            out=v_d[s][128 * lc:128 * (lc + 1),
                                       8 * 65 * jh:8 * 65 * (jh + 1)],
                            in_=vst[:])
                return run
            return [block(lc) for lc in range(NCH)]

        def attention_blocks(m, OT):
            sa, sb, sv = MHA_STREAMS[m]
            pend = {}

            def finalize(h, oc):
                po, co = 64 * (h % 2), (h // 2) * L
                rb = rbp.tile([1, L], f32, tag="rb", name="rb")
                rb_r = rbp.tile([1, L], f32r, tag="rbr", name="rbr")
                nc.vector.reciprocal(rb[0:1, :], oc[64:65, :])
                nc.vector.tensor_copy(rb_r[0:1, :], rb[0:1, :])
                for qh in range(2):
                    rb_ps = psum.tile([64, 512], f32, tag="fine",
                                      name="rb_ps", bufs=4)
                    nc.tensor.matmul(
                        rb_ps[:], lhsT=ones_r[0:1, :],
                        rhs=rb_r[0:1, 512 * qh:512 * (qh + 1)],
                        start=True, stop=True)
                    nc.vector.tensor_mul(
                        OT[po:po + 64, co + 512 * qh:co + 512 * (qh + 1)],
                        oc[0:64, 512 * qh:512 * (qh + 1)], rb_ps[:])

            def head(h):
                def run():
                    qcat = qkp.tile([128, L], bf16, tag="qcat", name="qcat")
                    kcat = qkp.tile([128, L], bf16, tag="kcat", name="kcat")
                    nc.sync.dma_start(
                        out=qcat[0:64, :], in_=qT_d[sa][64 * h:64 * h + 64, :])
                    nc.sync.dma_start(
                        out=qcat[64:128, :], in_=qT_d[sb][64 * h:64 * h + 64, :])
                    nc.sync.dma_start(
                        out=kcat[0:64, :], in_=kT_d[sa][64 * h:64 * h + 64, :])
                    nc.sync.dma_start(
                        out=kcat[64:128, :], in_=kT_d[sb][64 * h:64 * h + 64, :])
                    vext = qkp.tile([128, NCH * 65], bf16, tag="vext",
                                    name="vext", bufs=2)
                    vsrc = v_d[sv].rearrange("(c p) w -> p c w", p=128)
                    nc.sync.dma_start(
                        out=vext[:].rearrange("p (c w) -> p c w", w=65),
                        in_=vsrc[:, :, 65 * h:65 * (h + 1)])

                    o_ps = [psum.tile([65, 512], f32, tag="fine",
                                      name="ops", bufs=4) for _ in range(2)]
                    for c in range(NCH):
                        s_ps = psum.tile([128, L], f32, tag="scr", name="scr",
                                         bufs=2)
                        for qh in range(2):
                            nc.tensor.matmul(
                                s_ps[:, 512 * qh:512 * (qh + 1)],
                                lhsT=kcat[:, 128 * c:128 * (c + 1)],
                                rhs=qcat[:, 512 * qh:512 * (qh + 1)],
                                start=True, stop=True)
                        p_sb = ptp.tile([128, L], bf16, tag="p_sb", name="p_sb")
                        nc.scalar.activation(p_sb[:], s_ps[:], AF.Exp,
                                             scale=SCALE)
                        for qh in range(2):
                            nc.tensor.matmul(
                                o_ps[qh][0:65, :],
                                lhsT=vext[:, 65 * c:65 * (c + 1)],
                                rhs=p_sb[:, 512 * qh:512 * (qh + 1)],
                                start=(c == 0), stop=(c == NCH - 1))

                    # copy attention accumulator out of PSUM promptly
                    oc = stp.tile([65, L], f32, tag="stoc", name="oc")
                    for qh in range(2):
                        nc.vector.tensor_copy(
                            oc[:, 512 * qh:512 * (qh + 1)], o_ps[qh][:])
                    # finalize the PREVIOUS head here so the single-lane DVE
                    # row ops never head-of-line-block this head's copies
                    if pend:
                        (ph, poc), = pend.items()
                        finalize(ph, poc)
                        pend.clear()
                    pend[h] = oc
                return run

            def tail():
                (ph, poc), = pend.items()
                finalize(ph, poc)
                pend.clear()
            return [head(h) for h in range(H)] + [tail]

        def oproj_blocks(m, OT, wo_t):
            def block(qc):
                def run():
                    ost = stp.tile([128, L], f32, tag="stoc", name="ost")
                    op_ps = [psum.tile([128, 512], f32, tag="fine",
                                       name="pp", bufs=4) for _ in range(2)]
                    for c in range(NCH):
                        for dh in range(2):
                            nc.tensor.matmul(
                                op_ps[dh][:],
                                lhsT=OT[:, L * c + 128 * qc:L * c + 128 * (qc + 1)],
                                rhs=wo_t[:, D * c + 512 * dh:D * c + 512 * (dh + 1)],
                                start=(c == 0), stop=(c == NCH - 1))
                    for dh in range(2):
                        nc.vector.tensor_copy(
                            ost[:, 512 * dh:512 * (dh + 1)], op_ps[dh][:])
                    nc.scalar.dma_start(
                        out=out[128 * qc:128 * (qc + 1), m, :], in_=ost[:])
                return run
            return [block(qc) for qc in range(NCH)]

        for _rep in range(repeat):
            # ---------------- emission schedule ----------------
            OTs = {}

            def mk_ot(m):
                OTs[m] = xts.tile([128, NCH * L], bf16, tag="xts", name="OT")

            wq_t = load_w8(Wq8)
            load_xt(1)
            load_xt(2)
            load_xt(0)
            for s in (1, 2, 0):
                for b in proj_qk_blocks(wq_t, bq_t, s, qT_d, xt_tiles[s]):
                    b()
            wk_t = load_w8(Wk8)
            for s in (1, 2, 0):
                for b in proj_qk_blocks(wk_t, bk_t, s, kT_d, xt_tiles[s]):
                    b()
            wv_t = load_w8(Wv8)
            for b in proj_v_blocks(wv_t, 0, xt_tiles[0]):
                b()

            # A0 || (v1, v2): xt1/xt2 and Wv still resident
            mk_ot(0)

            def chain_emit():
                blocks = []
                for lc in range(NCH):
                    blocks.append(lambda lc=lc: proj_v_blocks(
                        wv_t, 1, xt_tiles[1])[lc]())
                for lc in range(NCH):
                    blocks.append(lambda lc=lc: proj_v_blocks(
                        wv_t, 2, xt_tiles[2])[lc]())
                return blocks

            _interleave(attention_blocks(0, OTs[0]), chain_emit())

            # A1 || (load Wo, oproj 0)
            mk_ot(1)
            wo_state = {}

            def o0_blocks():
                blocks = []

                def loadwo():
                    wo_state["w"] = load_wo(Wo)
                blocks.append(loadwo)
                for qc in range(NCH):
                    blocks.append(lambda qc=qc: oproj_blocks(
                        0, OTs[0], wo_state["w"])[qc]())
                return blocks

            _interleave(attention_blocks(1, OTs[1]), o0_blocks())

            # A2 || oproj 1
            mk_ot(2)
            _interleave(
                attention_blocks(2, OTs[2]),
                [lambda qc=qc: oproj_blocks(1, OTs[1], wo_state["w"])[qc]()
                 for qc in range(NCH)])

            for qc in range(NCH):
                oproj_blocks(2, OTs[2], wo_state["w"])[qc]()

        for p in reversed(cstack):
            p.release()

    _split_excess_waits(nc, max_waits=1)
    return nc


def get_program():
    if "nc" not in _CACHE:
        _CACHE["nc"] = _build_program()
    return _CACHE["nc"]


def _split_fp8(a, axis=0):
    """a (f32) -> (hi, lo) fp8e4m3 stacked on `axis` with hi + lo ~= a."""
    import ml_dtypes

    hi = a.astype(ml_dtypes.float8_e4m3)
    lo = (a - hi.astype(np.float32)).astype(ml_dtypes.float8_e4m3)
    return np.ascontiguousarray(np.stack([hi, lo], axis=axis))


def kernel(x, Wq, bq, Wk, bk, Wv, bv, Wo, bo):
    import ml_dtypes
    from concourse.bass_utils import run_bass_kernel_spmd

    nc = get_program()
    x = np.ascontiguousarray(np.asarray(x, dtype=np.float32))
    Wq = np.asarray(Wq, dtype=np.float32)
    Wk = np.asarray(Wk, dtype=np.float32)
    Wv = np.asarray(Wv, dtype=np.float32)
    ws = {
        "Wq8": _split_fp8(WSCL * Wq),
        "Wk8": _split_fp8(WSCL * Wk),
        "Wv8": _split_fp8(WSCL * Wv),
        "Wo": np.ascontiguousarray(np.asarray(Wo, dtype=np.float32).astype(ml_dtypes.bfloat16)),
        "bq": WSCL * np.asarray(bq, dtype=np.float32),
        "bk": WSCL * np.asarray(bk, dtype=np.float32),
    }
    bv = np.asarray(bv, dtype=np.float64)
    bo = np.asarray(bo, dtype=np.float64)
    in_maps = [
        dict(ws, xT8=_split_fp8(x[b].transpose(1, 2, 0), axis=1))
        for b in range(N_CORES)
    ]
    res = run_bass_kernel_spmd(nc, in_maps, list(range(N_CORES)))
    outp = np.stack([res.results[b]["out"] for b in range(N_CORES)], axis=0)
    # bv and bo fold into a constant output row: softmax rows sum to 1, so
    # attention(v + bv) = attention(v) + bv, and (o + bv) @ Wo + bo adds
    # (bv @ Wo + bo) to every output row.
    corr = bv @ np.asarray(Wo, dtype=np.float64) + bo
    if np.any(corr):
        outp = (outp.astype(np.float64) + corr[None, None, None, :]).astype(
            np.float32)
    return outp


# revision 20
# speedup vs baseline: 1.2735x; 1.0003x over previous
"""Trainium2 Bass kernel for nn_MultiHeadedAttention_41566693491186.

Three dual-score MHAs over the streams packed in x[:, :, 0:3, :], with shared
Wq/Wk/Wv/Wo. Data-parallel over batch B=8: one batch element per NeuronCore.

v3 design:
  - Host precomputes xT = x^T per stream and splits xT and 32*W{q,k,v} into
    fp8e4m3 (hi, lo) pairs: A ~= hi + lo with ~0.15% residual.  The nine
    input projections run as fp8 DoubleRow matmuls (2 k-tiles per pass)
    keeping hi*hi + hi*lo + lo*hi cross terms: 12 DR matmuls per [128,512]
    output tile vs 16 f32r matmuls, at near-bf16 accuracy.
  - The 32x weight scale cancels exactly: exp scale becomes 2^-14 (q and k
    both carry 32x), and the v ones-column is 32.0 so softmax denominators
    scale with the numerators.
  - All attention-side tensors (qT/kT/v spills, qcat/kcat/vext, p, OT) are
    bf16: same 1 cyc/row PE cost as f32r, half the DMA/SBUF.
  - Softmax denominators: exact DVE reciprocal + f32r K=1 broadcast matmul
    (1 cyc/row), normalize with one DVE mul into OT.
  - QK^T / PV / out-projection stay f32r-grade (bf16 inputs, f32 PSUM).

Per-core plan:
  P1  projections (interleaved with attention below):
        qT[s] = (32 x_s Wq)^T, kT[s] = (32 x_s Wk)^T  (W-stationary, [j, L])
        v[s]  =  32 x_s Wv    (x-stationary, out [L, j], interleaved with a
                               32.0 column per head for the denominators)
  P2  per (mha, head): S^T = kcat^T-chunks x qcat -> exp (ACT, scale 2^-14,
      bf16 out) -> PV accumulate o^T[d, q] + sums row -> DVE recip ->
      f32r broadcast -> DVE mul into OT (bf16).
  P3  out = OT^T @ Wo + bo  (OT-stationary, out [q, d_model]) -> DRAM.
"""

import sys

if "/opt/trn_rl_repo" not in sys.path:
    sys.path.insert(0, "/opt/trn_rl_repo")

import numpy as np

B, L, D = 8, 1024, 1024
H, DH = 16, 64
NCH = 8              # 128-sized chunks along D or L
SCALE = 0.0625 / 1024.0   # (1/sqrt(64)) * 0.5 / (32*32)
WSCL = 32.0
N_CORES = 8
# mha m reads (A, B, V) streams: q1/k1 from A, q2/k2 from B, v from V
MHA_STREAMS = ((1, 2, 0), (0, 2, 1), (0, 1, 2))

_CACHE = {}


def _split_excess_waits(nc, max_waits=1):
    """Stock neuronxcc walrus rejects instructions carrying more than
    `max_waits` semaphore waits; move excess onto same-engine NOPs."""
    import concourse.mybir as mybir

    for f in nc.m.functions:
        for bb in f.blocks:
            out = []
            changed = False
            for inst in bb.instructions:
                si = inst.sync_info
                waits = list(si.on_wait) if (si is not None and si.on_wait) else []
                if len(waits) > max_waits:
                    extra, keep = waits[:-max_waits], waits[-max_waits:]
                    k = 0
                    while extra:
                        chunk, extra = extra[:max_waits], extra[max_waits:]
                        nop = mybir.InstNoOp(
                            name=f"{inst.name}-ws{k}",
                            engine=inst.engine,
                            sync_info=mybir.SyncInfo(on_wait=chunk, on_update=[]),
                        )
                        out.append(nop)
                        k += 1
                    inst.sync_info = mybir.SyncInfo(
                        on_wait=keep,
                        on_update=list(si.on_update) if si.on_update else [],
                    )
                    changed = True
                out.append(inst)
            if changed:
                bb.instructions = out


def _interleave(*seqs):
    """Proportional merge of thunk lists, preserving within-list order."""
    items = []
    for si, seq in enumerate(seqs):
        n = len(seq)
        for i, thunk in enumerate(seq):
            items.append(((i + 0.5) / n, si, i, thunk))
    for _, _, _, t in sorted(items, key=lambda z: (z[0], z[1], z[2])):
        t()


def _build_program(repeat=1):
    import concourse.bass as bass
    import concourse.mybir as mybir
    import concourse.tile as tile

    f32 = mybir.dt.float32
    f32r = mybir.dt.float32r
    bf16 = mybir.dt.bfloat16
    f8 = mybir.dt.float8e4
    DR = mybir.MatmulPerfMode.DoubleRow
    AF = mybir.ActivationFunctionType

    nc = bass.Bass("TRN2", target_bir_lowering=False, debug=False)

    # hi/lo fp8 pairs, packed [2, D, L]: index 0 = hi, 1 = lo
    xT8 = nc.declare_dram_parameter("xT8", [3, 2, D, L], f8, isOutput=False)
    Wq8 = nc.declare_dram_parameter("Wq8", [2, D, D], f8, isOutput=False)
    Wk8 = nc.declare_dram_parameter("Wk8", [2, D, D], f8, isOutput=False)
    Wv8 = nc.declare_dram_parameter("Wv8", [2, D, D], f8, isOutput=False)
    Wo = nc.declare_dram_parameter("Wo", [D, D], bf16, isOutput=False)
    bq = nc.declare_dram_parameter("bq", [D], f32, isOutput=False)
    bk = nc.declare_dram_parameter("bk", [D], f32, isOutput=False)
    out = nc.declare_dram_parameter("out", [L, 3, D], f32, isOutput=True)

    # internal DRAM spill (bf16)
    qT_d = [nc.dram_tensor(f"qT{s}", [D, L], bf16) for s in range(3)]
    kT_d = [nc.dram_tensor(f"kT{s}", [D, L], bf16) for s in range(3)]
    # v: head h data at cols 65h..65h+64, 32.0 column at 65h+64
    v_d = [nc.dram_tensor(f"v{s}", [L, H * 65], bf16) for s in range(3)]

    with tile.TileContext(nc) as tc:
        cstack = []
        cp = tc.alloc_tile_pool(name="const", bufs=1)
        psum = tc.alloc_tile_pool(name="psum", bufs=1, space="PSUM")
        xts = tc.alloc_tile_pool(name="xts", bufs=4)
        cstack += [cp, psum, xts]

        cmisc = cp.tile([128, 96], f32, tag="cmisc", name="cmisc")
        ones64 = cmisc[:, 0:64]
        v32c = cmisc[:, 80:96]
        bq_t = cmisc[:, 64:72]
        bk_t = cmisc[:, 72:80]
        nc.gpsimd.memset(ones64, 1.0)
        nc.gpsimd.memset(v32c, WSCL)
        onesr = cp.tile([1, 64], f32r, tag="onesr", name="onesr")
        nc.vector.tensor_copy(onesr[:], ones64[0:1, :])
        ones_r = onesr
        nc.sync.dma_start(out=bq_t, in_=bq.rearrange("(c p) -> p c", p=128))
        nc.sync.dma_start(out=bk_t, in_=bk.rearrange("(c p) -> p c", p=128))

        # ---------------- xT loads (fp8 hi+lo, host pre-transposed) ---------
        xt_tiles = {}

        def load_xt(s):
            # [128, hilo, c, l] fp8
            xt = xts.tile([128, 2, NCH, L], f8, tag="xts", name=f"xt{s}")
            src = xT8[s].rearrange("t (c p) l -> p t c l", p=128)
            for t in range(2):
                nc.sync.dma_start(out=xt[:, t, 0:4, :], in_=src[:, t, 0:4, :])
                nc.scalar.dma_start(out=xt[:, t, 4:8, :], in_=src[:, t, 4:8, :])
            xt_tiles[s] = xt

        # ---------------- shared pools for P1/P2/P3 ----------------
        wrp = tc.alloc_tile_pool(name="wrp", bufs=1)
        stp = tc.alloc_tile_pool(name="stp", bufs=5)
        qkp = tc.alloc_tile_pool(name="qkp", bufs=3)
        ptp = tc.alloc_tile_pool(name="ptp", bufs=3)
        rbp = tc.alloc_tile_pool(name="rbp", bufs=2)
        cstack += [wrp, stp, qkp, ptp, rbp]

        def load_w8(Wsrc):
            # [128, hilo, c, d] fp8
            w_t = wrp.tile([128, 2, NCH, D], f8, tag="W8", name="W8")
            src = Wsrc.rearrange("t (c p) d -> p t c d", p=128)
            for t in range(2):
                nc.sync.dma_start(out=w_t[:, t, 0:4, :], in_=src[:, t, 0:4, :])
                nc.scalar.dma_start(out=w_t[:, t, 4:8, :], in_=src[:, t, 4:8, :])
            return w_t

        def load_wo(Wsrc):
            w_t = wrp.tile([128, NCH * D], bf16, tag="Wor", name="Wor")
            d3 = w_t[:].rearrange("p (c d) -> p c d", d=D)
            s3 = Wsrc.rearrange("(c p) d -> p c d", p=128)
            nc.sync.dma_start(out=d3[:, 0:4, :], in_=s3[:, 0:4, :])
            nc.scalar.dma_start(out=d3[:, 4:8, :], in_=s3[:, 4:8, :])
            return w_t

        def dr_products(emit, w_t, xt):
            """12 DoubleRow matmuls accumulating hi*hi + hi*lo + lo*hi over
            4 chunk-pairs; emit(lhs_sel, rhs_sel, t, first, last)."""
            combos = ((0, 0), (0, 1), (1, 0))
            n = 0
            for t in range(4):
                for (wi, xi) in combos:
                    n += 1
                    emit(wi, xi, t, n == 1, n == 12)

        def proj_qk_blocks(w_t, b_t, s, outd, xt):
            # out [j, L] = (32 x_s W)^T, one block per jc
            def block(jc):
                def run():
                    st = stp.tile([128, L], bf16, tag="stq", name="st")
                    ps = [psum.tile([128, 512], f32, tag="fine",
                                    name="pp", bufs=4) for _ in range(2)]

                    def emit(wi, xi, t, first, last):
                        for lh in range(2):
                            nc.tensor.matmul(
                                ps[lh][:],
                                lhsT=w_t[:, wi, 2 * t:2 * t + 2,
                                         128 * jc:128 * (jc + 1)],
                                rhs=xt[:, xi, 2 * t:2 * t + 2,
                                       512 * lh:512 * (lh + 1)],
                                start=first, stop=last, perf_mode=DR)
                    dr_products(emit, w_t, xt)
                    for lh in range(2):
                        nc.vector.tensor_scalar_add(
                            st[:, 512 * lh:512 * (lh + 1)], ps[lh][:],
                            b_t[:, jc:jc + 1])
                    nc.scalar.dma_start(
                        out=outd[s][128 * jc:128 * (jc + 1), :], in_=st[:])
                return run
            return [block(jc) for jc in range(NCH)]

        def proj_v_blocks(w_t, s, xt):
            # out [L, j] with interleaved 32.0 columns, one block per lc
            def block(lc):
                def run():
                    ps = [psum.tile([128, 512], f32, tag="fine",
                                    name="pp", bufs=4) for _ in range(2)]

                    def emit(wi, xi, t, first, last):
                        for jh in range(2):
                            nc.tensor.matmul(
                                ps[jh][:],
                                lhsT=xt[:, xi, 2 * t:2 * t + 2,
                                        128 * lc:128 * (lc + 1)],
                                rhs=w_t[:, wi, 2 * t:2 * t + 2,
                                        512 * jh:512 * (jh + 1)],
                                start=first, stop=last, perf_mode=DR)
                    dr_products(emit, w_t, xt)
                    for jh in range(2):
                        vst = stp.tile([128, 8 * 65], bf16, tag="stv", name="vst")
                        r = vst[:].rearrange("p (h w) -> p h w", w=65)
                        q3 = ps[jh][:].rearrange(
                            "p (h w) -> p h w", w=64)
                        nc.vector.tensor_copy(r[:, :, 0:64], q3)
                        nc.vector.tensor_copy(
                            r[:, :, 64:65].squeeze(2), v32c[:, 0:8])
                        nc.scalar.dma_start(
                            out=v_d[s][128 * lc:128 * (lc + 1),
                                       8 * 65 * jh:8 * 65 * (jh + 1)],
                            in_=vst[:])
                return run
            return [block(lc) for lc in range(NCH)]

        def attention_blocks(m, OT):
            sa, sb, sv = MHA_STREAMS[m]
            pend = {}

            def finalize(h, oc):
                po, co = 64 * (h % 2), (h // 2) * L
                rb = rbp.tile([1, L], f32, tag="rb", name="rb")
                rb_r = rbp.tile([1, L], f32r, tag="rbr", name="rbr")
                nc.vector.reciprocal(rb[0:1, :], oc[64:65, :])
                nc.vector.tensor_copy(rb_r[0:1, :], rb[0:1, :])
                for qh in range(2):
                    rb_ps = psum.tile([64, 512], f32, tag="fine",
                                      name="rb_ps", bufs=4)
                    nc.tensor.matmul(
                        rb_ps[:], lhsT=ones_r[0:1, :],
                        rhs=rb_r[0:1, 512 * qh:512 * (qh + 1)],
                        start=True, stop=True)
                    nc.vector.tensor_mul(
                        OT[po:po + 64, co + 512 * qh:co + 512 * (qh + 1)],
                        oc[0:64, 512 * qh:512 * (qh + 1)], rb_ps[:])

            def head(h):
                def run():
                    qcat = qkp.tile([128, L], bf16, tag="qcat", name="qcat")
                    kcat = qkp.tile([128, L], bf16, tag="kcat", name="kcat")
                    nc.sync.dma_start(
                        out=qcat[0:64, :], in_=qT_d[sa][64 * h:64 * h + 64, :])
                    nc.sync.dma_start(
                        out=qcat[64:128, :], in_=qT_d[sb][64 * h:64 * h + 64, :])
                    nc.sync.dma_start(
                        out=kcat[0:64, :], in_=kT_d[sa][64 * h:64 * h + 64, :])
                    nc.sync.dma_start(
                        out=kcat[64:128, :], in_=kT_d[sb][64 * h:64 * h + 64, :])
                    vext = qkp.tile([128, NCH * 65], bf16, tag="vext",
                                    name="vext", bufs=2)
                    vsrc = v_d[sv].rearrange("(c p) w -> p c w", p=128)
                    nc.sync.dma_start(
                        out=vext[:].rearrange("p (c w) -> p c w", w=65),
                        in_=vsrc[:, :, 65 * h:65 * (h + 1)])

                    o_ps = [psum.tile([65, 512], f32, tag="fine",
                                      name="ops", bufs=4) for _ in range(2)]

                    # software pipeline: emit QK(c+1) before PV(c) so the
                    # in-order PE queue never head-of-line blocks on exp(c)
                    def qk(c):
                        s_ps = psum.tile([128, L], f32, tag="scr", name="scr",
                                         bufs=2)
                        for qh in range(2):
                            nc.tensor.matmul(
                                s_ps[:, 512 * qh:512 * (qh + 1)],
                                lhsT=kcat[:, 128 * c:128 * (c + 1)],
                                rhs=qcat[:, 512 * qh:512 * (qh + 1)],
                                start=True, stop=True)
                        p_sb = ptp.tile([128, L], bf16, tag="p_sb", name="p_sb")
                        nc.scalar.activation(p_sb[:], s_ps[:], AF.Exp,
                                             scale=SCALE)
                        return p_sb

                    def pv(c, p_sb):
                        for qh in range(2):
                            nc.tensor.matmul(
                                o_ps[qh][0:65, :],
                                lhsT=vext[:, 65 * c:65 * (c + 1)],
                                rhs=p_sb[:, 512 * qh:512 * (qh + 1)],
                                start=(c == 0), stop=(c == NCH - 1))

                    prev = qk(0)
                    for c in range(1, NCH):
                        cur = qk(c)
                        pv(c - 1, prev)
                        prev = cur
                    pv(NCH - 1, prev)

                    # copy attention accumulator out of PSUM promptly
                    oc = stp.tile([65, L], f32, tag="stoc", name="oc")
                    for qh in range(2):
                        nc.vector.tensor_copy(
                            oc[:, 512 * qh:512 * (qh + 1)], o_ps[qh][:])
                    # finalize the PREVIOUS head here so the single-lane DVE
                    # row ops never head-of-line-block this head's copies
                    if pend:
                        (ph, poc), = pend.items()
                        finalize(ph, poc)
                        pend.clear()
                    pend[h] = oc
                return run

            def tail():
                (ph, poc), = pend.items()
                finalize(ph, poc)
                pend.clear()
            return [head(h) for h in range(H)] + [tail]

        def oproj_blocks(m, OT, wo_t):
            def block(qc):
                def run():
                    ost = stp.tile([128, L], f32, tag="stoc", name="ost")
                    op_ps = [psum.tile([128, 512], f32, tag="fine",
                                       name="pp", bufs=4) for _ in range(2)]
                    for c in range(NCH):
                        for dh in range(2):
                            nc.tensor.matmul(
                                op_ps[dh][:],
                                lhsT=OT[:, L * c + 128 * qc:L * c + 128 * (qc + 1)],
                                rhs=wo_t[:, D * c + 512 * dh:D * c + 512 * (dh + 1)],
                                start=(c == 0), stop=(c == NCH - 1))
                    for dh in range(2):
                        nc.vector.tensor_copy(
                            ost[:, 512 * dh:512 * (dh + 1)], op_ps[dh][:])
                    nc.scalar.dma_start(
                        out=out[128 * qc:128 * (qc + 1), m, :], in_=ost[:])
                return run
            return [block(qc) for qc in range(NCH)]

        for _rep in range(repeat):
            # ---------------- emission schedule ----------------
            OTs = {}

            def mk_ot(m):
                OTs[m] = xts.tile([128, NCH * L], bf16, tag="xts", name="OT")

            wq_t = load_w8(Wq8)
            load_xt(1)
            load_xt(2)
            load_xt(0)
            for s in (1, 2, 0):
                for b in proj_qk_blocks(wq_t, bq_t, s, qT_d, xt_tiles[s]):
                    b()
            wk_t = load_w8(Wk8)
            for s in (1, 2, 0):
                for b in proj_qk_blocks(wk_t, bk_t, s, kT_d, xt_tiles[s]):
                    b()
            wv_t = load_w8(Wv8)
            for b in proj_v_blocks(wv_t, 0, xt_tiles[0]):
                b()

            # A0 || (v1, v2): xt1/xt2 and Wv still resident
            mk_ot(0)

            def chain_emit():
                blocks = []
                for lc in range(NCH):
                    blocks.append(lambda lc=lc: proj_v_blocks(
                        wv_t, 1, xt_tiles[1])[lc]())
                for lc in range(NCH):
                    blocks.append(lambda lc=lc: proj_v_blocks(
                        wv_t, 2, xt_tiles[2])[lc]())
                return blocks

            _interleave(attention_blocks(0, OTs[0]), chain_emit())

            # A1 || (load Wo, oproj 0)
            mk_ot(1)
            wo_state = {}

            def o0_blocks():
                blocks = []

                def loadwo():
                    wo_state["w"] = load_wo(Wo)
                blocks.append(loadwo)
                for qc in range(NCH):
                    blocks.append(lambda qc=qc: oproj_blocks(
                        0, OTs[0], wo_state["w"])[qc]())
                return blocks

            _interleave(attention_blocks(1, OTs[1]), o0_blocks())

            # A2 || oproj 1
            mk_ot(2)
            _interleave(
                attention_blocks(2, OTs[2]),
                [lambda qc=qc: oproj_blocks(1, OTs[1], wo_state["w"])[qc]()
                 for qc in range(NCH)])

            for qc in range(NCH):
                oproj_blocks(2, OTs[2], wo_state["w"])[qc]()

        for p in reversed(cstack):
            p.release()

    _split_excess_waits(nc, max_waits=1)
    return nc


def get_program():
    if "nc" not in _CACHE:
        _CACHE["nc"] = _build_program()
    return _CACHE["nc"]


def _split_fp8(a, axis=0):
    """a (f32) -> (hi, lo) fp8e4m3 stacked on `axis` with hi + lo ~= a."""
    import ml_dtypes

    hi = a.astype(ml_dtypes.float8_e4m3)
    lo = (a - hi.astype(np.float32)).astype(ml_dtypes.float8_e4m3)
    return np.ascontiguousarray(np.stack([hi, lo], axis=axis))


def kernel(x, Wq, bq, Wk, bk, Wv, bv, Wo, bo):
    import ml_dtypes
    from concourse.bass_utils import run_bass_kernel_spmd

    nc = get_program()
    x = np.ascontiguousarray(np.asarray(x, dtype=np.float32))
    Wq = np.asarray(Wq, dtype=np.float32)
    Wk = np.asarray(Wk, dtype=np.float32)
    Wv = np.asarray(Wv, dtype=np.float32)
    ws = {
        "Wq8": _split_fp8(WSCL * Wq),
        "Wk8": _split_fp8(WSCL * Wk),
        "Wv8": _split_fp8(WSCL * Wv),
        "Wo": np.ascontiguousarray(np.asarray(Wo, dtype=np.float32).astype(ml_dtypes.bfloat16)),
        "bq": WSCL * np.asarray(bq, dtype=np.float32),
        "bk": WSCL * np.asarray(bk, dtype=np.float32),
    }
    bv = np.asarray(bv, dtype=np.float64)
    bo = np.asarray(bo, dtype=np.float64)
    in_maps = [
        dict(ws, xT8=_split_fp8(x[b].transpose(1, 2, 0), axis=1))
        for b in range(N_CORES)
    ]
    res = run_bass_kernel_spmd(nc, in_maps, list(range(N_CORES)))
    outp = np.stack([res.results[b]["out"] for b in range(N_CORES)], axis=0)
    # bv and bo fold into a constant output row: softmax rows sum to 1, so
    # attention(v + bv) = attention(v) + bv, and (o + bv) @ Wo + bo adds
    # (bv @ Wo + bo) to every output row.
    corr = bv @ np.asarray(Wo, dtype=np.float64) + bo
    if np.any(corr):
        outp = (outp.astype(np.float64) + corr[None, None, None, :]).astype(
            np.float32)
    return outp


# revision 21
# speedup vs baseline: 1.3287x; 1.0433x over previous
"""Trainium2 Bass kernel for nn_MultiHeadedAttention_41566693491186.

Three dual-score MHAs over the streams packed in x[:, :, 0:3, :], with shared
Wq/Wk/Wv/Wo. Data-parallel over batch B=8: one batch element per NeuronCore.

v3 design:
  - Host precomputes xT = x^T per stream and splits xT and 32*W{q,k,v} into
    fp8e4m3 (hi, lo) pairs: A ~= hi + lo with ~0.15% residual.  The nine
    input projections run as fp8 DoubleRow matmuls (2 k-tiles per pass)
    keeping hi*hi + hi*lo + lo*hi cross terms: 12 DR matmuls per [128,512]
    output tile vs 16 f32r matmuls, at near-bf16 accuracy.
  - The 32x weight scale cancels exactly: exp scale becomes 2^-14 (q and k
    both carry 32x), and the v ones-column is 32.0 so softmax denominators
    scale with the numerators.
  - All attention-side tensors (qT/kT/v spills, qcat/kcat/vext, p, OT) are
    bf16: same 1 cyc/row PE cost as f32r, half the DMA/SBUF.
  - Softmax denominators: exact DVE reciprocal + f32r K=1 broadcast matmul
    (1 cyc/row), normalize with one DVE mul into OT.
  - QK^T / PV / out-projection stay f32r-grade (bf16 inputs, f32 PSUM).

Per-core plan:
  P1  projections (interleaved with attention below):
        qT[s] = (32 x_s Wq)^T, kT[s] = (32 x_s Wk)^T  (W-stationary, [j, L])
        v[s]  =  32 x_s Wv    (x-stationary, out [L, j], interleaved with a
                               32.0 column per head for the denominators)
  P2  per (mha, head): S^T = kcat^T-chunks x qcat -> exp (ACT, scale 2^-14,
      bf16 out) -> PV accumulate o^T[d, q] + sums row -> DVE recip ->
      f32r broadcast -> DVE mul into OT (bf16).
  P3  out = OT^T @ Wo + bo  (OT-stationary, out [q, d_model]) -> DRAM.
"""

import sys

if "/opt/trn_rl_repo" not in sys.path:
    sys.path.insert(0, "/opt/trn_rl_repo")

import numpy as np

B, L, D = 8, 1024, 1024
H, DH = 16, 64
NCH = 8              # 128-sized chunks along D or L
SCALE = 0.0625 / 1024.0   # (1/sqrt(64)) * 0.5 / (32*32)
WSCL = 32.0
N_CORES = 8
# mha m reads (A, B, V) streams: q1/k1 from A, q2/k2 from B, v from V
MHA_STREAMS = ((1, 2, 0), (0, 2, 1), (0, 1, 2))

_CACHE = {}


def _split_excess_waits(nc, max_waits=1):
    """Stock neuronxcc walrus rejects instructions carrying more than
    `max_waits` semaphore waits; move excess onto same-engine NOPs."""
    import concourse.mybir as mybir

    for f in nc.m.functions:
        for bb in f.blocks:
            out = []
            changed = False
            for inst in bb.instructions:
                si = inst.sync_info
                waits = list(si.on_wait) if (si is not None and si.on_wait) else []
                if len(waits) > max_waits:
                    extra, keep = waits[:-max_waits], waits[-max_waits:]
                    k = 0
                    while extra:
                        chunk, extra = extra[:max_waits], extra[max_waits:]
                        nop = mybir.InstNoOp(
                            name=f"{inst.name}-ws{k}",
                            engine=inst.engine,
                            sync_info=mybir.SyncInfo(on_wait=chunk, on_update=[]),
                        )
                        out.append(nop)
                        k += 1
                    inst.sync_info = mybir.SyncInfo(
                        on_wait=keep,
                        on_update=list(si.on_update) if si.on_update else [],
                    )
                    changed = True
                out.append(inst)
            if changed:
                bb.instructions = out


def _interleave(*seqs):
    """Proportional merge of thunk lists, preserving within-list order."""
    items = []
    for si, seq in enumerate(seqs):
        n = len(seq)
        for i, thunk in enumerate(seq):
            items.append(((i + 0.5) / n, si, i, thunk))
    for _, _, _, t in sorted(items, key=lambda z: (z[0], z[1], z[2])):
        t()


def _build_program(repeat=1):
    import concourse.bass as bass
    import concourse.mybir as mybir
    import concourse.tile as tile

    f32 = mybir.dt.float32
    f32r = mybir.dt.float32r
    bf16 = mybir.dt.bfloat16
    f8 = mybir.dt.float8e4
    DR = mybir.MatmulPerfMode.DoubleRow
    AF = mybir.ActivationFunctionType

    nc = bass.Bass("TRN2", target_bir_lowering=False, debug=False)

    # hi/lo fp8 pairs, packed [2, D, L]: index 0 = hi, 1 = lo
    xT8 = nc.declare_dram_parameter("xT8", [3, 2, D, L], f8, isOutput=False)
    Wq8 = nc.declare_dram_parameter("Wq8", [2, D, D], f8, isOutput=False)
    Wk8 = nc.declare_dram_parameter("Wk8", [2, D, D], f8, isOutput=False)
    Wv8 = nc.declare_dram_parameter("Wv8", [2, D, D], f8, isOutput=False)
    Wo = nc.declare_dram_parameter("Wo", [D, D], bf16, isOutput=False)
    bq = nc.declare_dram_parameter("bq", [D], f32, isOutput=False)
    bk = nc.declare_dram_parameter("bk", [D], f32, isOutput=False)
    out = nc.declare_dram_parameter("out", [L, 3, D], f32, isOutput=True)

    # internal DRAM spill (bf16)
    qT_d = [nc.dram_tensor(f"qT{s}", [D, L], bf16) for s in range(3)]
    kT_d = [nc.dram_tensor(f"kT{s}", [D, L], bf16) for s in range(3)]
    # v: head h data at cols 65h..65h+64, 32.0 column at 65h+64
    v_d = [nc.dram_tensor(f"v{s}", [L, H * 65], bf16) for s in range(3)]

    with tile.TileContext(nc) as tc:
        cstack = []
        cp = tc.alloc_tile_pool(name="const", bufs=1)
        psum = tc.alloc_tile_pool(name="psum", bufs=1, space="PSUM")
        xts = tc.alloc_tile_pool(name="xts", bufs=4)
        cstack += [cp, psum, xts]

        cmisc = cp.tile([128, 96], f32, tag="cmisc", name="cmisc")
        ones64 = cmisc[:, 0:64]
        v32c = cmisc[:, 80:96]
        bq_t = cmisc[:, 64:72]
        bk_t = cmisc[:, 72:80]
        nc.gpsimd.memset(ones64, 1.0)
        nc.gpsimd.memset(v32c, WSCL)
        onesr = cp.tile([1, 64], f32r, tag="onesr", name="onesr")
        nc.vector.tensor_copy(onesr[:], ones64[0:1, :])
        ones_r = onesr
        nc.sync.dma_start(out=bq_t, in_=bq.rearrange("(c p) -> p c", p=128))
        nc.sync.dma_start(out=bk_t, in_=bk.rearrange("(c p) -> p c", p=128))

        # ---------------- xT loads (fp8 hi+lo, host pre-transposed) ---------
        xt_tiles = {}

        def load_xt(s):
            # [128, hilo, c, l] fp8
            xt = xts.tile([128, 2, NCH, L], f8, tag="xts", name=f"xt{s}")
            src = xT8[s].rearrange("t (c p) l -> p t c l", p=128)
            for t in range(2):
                nc.sync.dma_start(out=xt[:, t, 0:4, :], in_=src[:, t, 0:4, :])
                nc.scalar.dma_start(out=xt[:, t, 4:8, :], in_=src[:, t, 4:8, :])
            xt_tiles[s] = xt

        # ---------------- shared pools for P1/P2/P3 ----------------
        wrp = tc.alloc_tile_pool(name="wrp", bufs=1)
        stp = tc.alloc_tile_pool(name="stp", bufs=5)
        qkp = tc.alloc_tile_pool(name="qkp", bufs=3)
        ptp = tc.alloc_tile_pool(name="ptp", bufs=3)
        rbp = tc.alloc_tile_pool(name="rbp", bufs=2)
        cstack += [wrp, stp, qkp, ptp, rbp]

        def load_w8(Wsrc):
            # [128, hilo, c, d] fp8
            w_t = wrp.tile([128, 2, NCH, D], f8, tag="W8", name="W8", bufs=2)
            src = Wsrc.rearrange("t (c p) d -> p t c d", p=128)
            for t in range(2):
                nc.sync.dma_start(out=w_t[:, t, 0:4, :], in_=src[:, t, 0:4, :])
                nc.scalar.dma_start(out=w_t[:, t, 4:8, :], in_=src[:, t, 4:8, :])
            return w_t

        def load_wo(Wsrc):
            w_t = wrp.tile([128, NCH * D], bf16, tag="Wor", name="Wor")
            d3 = w_t[:].rearrange("p (c d) -> p c d", d=D)
            s3 = Wsrc.rearrange("(c p) d -> p c d", p=128)
            nc.sync.dma_start(out=d3[:, 0:4, :], in_=s3[:, 0:4, :])
            nc.scalar.dma_start(out=d3[:, 4:8, :], in_=s3[:, 4:8, :])
            return w_t

        def dr_products(emit, w_t, xt):
            """12 DoubleRow matmuls accumulating hi*hi + hi*lo + lo*hi over
            4 chunk-pairs; emit(lhs_sel, rhs_sel, t, first, last)."""
            combos = ((0, 0), (0, 1), (1, 0))
            n = 0
            for t in range(4):
                for (wi, xi) in combos:
                    n += 1
                    emit(wi, xi, t, n == 1, n == 12)

        def proj_qk_blocks(w_t, b_t, s, outd, xt):
            # out [j, L] = (32 x_s W)^T, one block per jc
            def block(jc):
                def run():
                    st = stp.tile([128, L], bf16, tag="stq", name="st")
                    for lh in range(2):
                        ps = psum.tile([128, 512], f32, tag="pp",
                                       name="pp", bufs=2)

                        def emit(wi, xi, t, first, last, lh=lh, ps=ps):
                            nc.tensor.matmul(
                                ps[:],
                                lhsT=w_t[:, wi, 2 * t:2 * t + 2,
                                         128 * jc:128 * (jc + 1)],
                                rhs=xt[:, xi, 2 * t:2 * t + 2,
                                       512 * lh:512 * (lh + 1)],
                                start=first, stop=last, perf_mode=DR)
                        dr_products(emit, w_t, xt)
                        nc.vector.tensor_scalar_add(
                            st[:, 512 * lh:512 * (lh + 1)], ps[:],
                            b_t[:, jc:jc + 1])
                    nc.scalar.dma_start(
                        out=outd[s][128 * jc:128 * (jc + 1), :], in_=st[:])
                return run
            return [block(jc) for jc in range(NCH)]

        def proj_v_blocks(w_t, s, xt):
            # out [L, j] with interleaved 32.0 columns, one block per lc
            def block(lc):
                def run():
                    for jh in range(2):
                        ps = psum.tile([128, 512], f32, tag="pp",
                                       name="pp", bufs=2)

                        def emit(wi, xi, t, first, last, jh=jh, ps=ps):
                            nc.tensor.matmul(
                                ps[:],
                                lhsT=xt[:, xi, 2 * t:2 * t + 2,
                                        128 * lc:128 * (lc + 1)],
                                rhs=w_t[:, wi, 2 * t:2 * t + 2,
                                        512 * jh:512 * (jh + 1)],
                                start=first, stop=last, perf_mode=DR)
                        dr_products(emit, w_t, xt)
                        vst = stp.tile([128, 8 * 65], bf16, tag="stv", name="vst")
                        r = vst[:].rearrange("p (h w) -> p h w", w=65)
                        q3 = ps[:].rearrange("p (h w) -> p h w", w=64)
                        nc.vector.tensor_copy(r[:, :, 0:64], q3)
                        nc.vector.tensor_copy(
                            r[:, :, 64:65].squeeze(2), v32c[:, 0:8])
                        nc.scalar.dma_start(
                            out=v_d[s][128 * lc:128 * (lc + 1),
                                       8 * 65 * jh:8 * 65 * (jh + 1)],
                            in_=vst[:])
                return run
            return [block(lc) for lc in range(NCH)]

        def attention_blocks(m, OT):
            sa, sb, sv = MHA_STREAMS[m]
            pend = {}

            def finalize(h, oc):
                po, co = 64 * (h % 2), (h // 2) * L
                rb = rbp.tile([1, L], f32, tag="rb", name="rb")
                rb_r = rbp.tile([1, L], f32r, tag="rbr", name="rbr")
                nc.vector.reciprocal(rb[0:1, :], oc[64:65, :])
                nc.vector.tensor_copy(rb_r[0:1, :], rb[0:1, :])
                for qh in range(2):
                    rb_ps = psum.tile([64, 512], f32, tag="pp",
                                      name="rb_ps", bufs=2)
                    nc.tensor.matmul(
                        rb_ps[:], lhsT=ones_r[0:1, :],
                        rhs=rb_r[0:1, 512 * qh:512 * (qh + 1)],
                        start=True, stop=True)
                    nc.vector.tensor_mul(
                        OT[po:po + 64, co + 512 * qh:co + 512 * (qh + 1)],
                        oc[0:64, 512 * qh:512 * (qh + 1)], rb_ps[:])

            def head(h):
                def run():
                    qcat = qkp.tile([128, L], bf16, tag="qcat", name="qcat", bufs=4)
                    kcat = qkp.tile([128, L], bf16, tag="kcat", name="kcat", bufs=4)
                    nc.sync.dma_start(
                        out=qcat[0:64, :], in_=qT_d[sa][64 * h:64 * h + 64, :])
                    nc.sync.dma_start(
                        out=qcat[64:128, :], in_=qT_d[sb][64 * h:64 * h + 64, :])
                    nc.sync.dma_start(
                        out=kcat[0:64, :], in_=kT_d[sa][64 * h:64 * h + 64, :])
                    nc.sync.dma_start(
                        out=kcat[64:128, :], in_=kT_d[sb][64 * h:64 * h + 64, :])
                    vext = qkp.tile([128, NCH * 65], bf16, tag="vext",
                                    name="vext", bufs=3)
                    vsrc = v_d[sv].rearrange("(c p) w -> p c w", p=128)
                    nc.sync.dma_start(
                        out=vext[:].rearrange("p (c w) -> p c w", w=65),
                        in_=vsrc[:, :, 65 * h:65 * (h + 1)])

                    o_ps = [psum.tile([65, 512], f32, tag="ops",
                                      name="ops", bufs=2) for _ in range(2)]

                    # software pipeline: emit QK(c+1) before PV(c) so the
                    # in-order PE queue never head-of-line blocks on exp(c)
                    def qk(c):
                        s_ps = psum.tile([128, L], f32, tag="scr", name="scr",
                                         bufs=2)
                        for qh in range(2):
                            nc.tensor.matmul(
                                s_ps[:, 512 * qh:512 * (qh + 1)],
                                lhsT=kcat[:, 128 * c:128 * (c + 1)],
                                rhs=qcat[:, 512 * qh:512 * (qh + 1)],
                                start=True, stop=True)
                        p_sb = ptp.tile([128, L], bf16, tag="p_sb", name="p_sb")
                        nc.scalar.activation(p_sb[:], s_ps[:], AF.Exp,
                                             scale=SCALE)
                        return p_sb

                    def pv(c, p_sb):
                        for qh in range(2):
                            nc.tensor.matmul(
                                o_ps[qh][0:65, :],
                                lhsT=vext[:, 65 * c:65 * (c + 1)],
                                rhs=p_sb[:, 512 * qh:512 * (qh + 1)],
                                start=(c == 0), stop=(c == NCH - 1))

                    prev = qk(0)
                    for c in range(1, NCH):
                        cur = qk(c)
                        pv(c - 1, prev)
                        prev = cur
                    pv(NCH - 1, prev)

                    # copy attention accumulator out of PSUM promptly
                    oc = stp.tile([65, L], f32, tag="stoc", name="oc")
                    for qh in range(2):
                        nc.vector.tensor_copy(
                            oc[:, 512 * qh:512 * (qh + 1)], o_ps[qh][:])
                    # finalize the PREVIOUS head here so the single-lane DVE
                    # row ops never head-of-line-block this head's copies
                    if pend:
                        (ph, poc), = pend.items()
                        finalize(ph, poc)
                        pend.clear()
                    pend[h] = oc
                return run

            def tail():
                (ph, poc), = pend.items()
                finalize(ph, poc)
                pend.clear()
            return [head(h) for h in range(H)] + [tail]

        def oproj_blocks(m, OT, wo_t):
            def block(qc):
                def run():
                    ost = stp.tile([128, L], f32, tag="stoc", name="ost")
                    for dh in range(2):
                        op_ps = psum.tile([128, 512], f32, tag="pp",
                                          name="pp", bufs=2)
                        for c in range(NCH):
                            nc.tensor.matmul(
                                op_ps[:],
                                lhsT=OT[:, L * c + 128 * qc:L * c + 128 * (qc + 1)],
                                rhs=wo_t[:, D * c + 512 * dh:D * c + 512 * (dh + 1)],
                                start=(c == 0), stop=(c == NCH - 1))
                        nc.vector.tensor_copy(
                            ost[:, 512 * dh:512 * (dh + 1)], op_ps[:])
                    nc.scalar.dma_start(
                        out=out[128 * qc:128 * (qc + 1), m, :], in_=ost[:])
                return run
            return [block(qc) for qc in range(NCH)]

        for _rep in range(repeat):
            # ---------------- emission schedule ----------------
            OTs = {}

            def mk_ot(m):
                OTs[m] = xts.tile([128, NCH * L], bf16, tag="xts", name="OT")

            wq_t = load_w8(Wq8)
            load_xt(1)
            load_xt(2)
            load_xt(0)
            for b in proj_qk_blocks(wq_t, bq_t, 1, qT_d, xt_tiles[1]):
                b()
            wk_t = load_w8(Wk8)
            for s in (2, 0):
                for b in proj_qk_blocks(wq_t, bq_t, s, qT_d, xt_tiles[s]):
                    b()
            for b in proj_qk_blocks(wk_t, bk_t, 1, kT_d, xt_tiles[1]):
                b()
            wv_t = load_w8(Wv8)
            for s in (2, 0):
                for b in proj_qk_blocks(wk_t, bk_t, s, kT_d, xt_tiles[s]):
                    b()
            for b in proj_v_blocks(wv_t, 0, xt_tiles[0]):
                b()

            # A0 || (v1, v2): xt1/xt2 and Wv still resident
            mk_ot(0)

            def chain_emit():
                blocks = []
                for lc in range(NCH):
                    blocks.append(lambda lc=lc: proj_v_blocks(
                        wv_t, 1, xt_tiles[1])[lc]())
                for lc in range(NCH):
                    blocks.append(lambda lc=lc: proj_v_blocks(
                        wv_t, 2, xt_tiles[2])[lc]())
                return blocks

            _interleave(attention_blocks(0, OTs[0]), chain_emit())

            # A1 || (load Wo, oproj 0)
            mk_ot(1)
            wo_state = {}

            def o0_blocks():
                blocks = []

                def loadwo():
                    wo_state["w"] = load_wo(Wo)
                blocks.append(loadwo)
                for qc in range(NCH):
                    blocks.append(lambda qc=qc: oproj_blocks(
                        0, OTs[0], wo_state["w"])[qc]())
                return blocks

            _interleave(attention_blocks(1, OTs[1]), o0_blocks())

            # A2 || oproj 1
            mk_ot(2)
            _interleave(
                attention_blocks(2, OTs[2]),
                [lambda qc=qc: oproj_blocks(1, OTs[1], wo_state["w"])[qc]()
                 for qc in range(NCH)])

            for qc in range(NCH):
                oproj_blocks(2, OTs[2], wo_state["w"])[qc]()

        for p in reversed(cstack):
            p.release()

    _split_excess_waits(nc, max_waits=1)
    return nc


def get_program():
    if "nc" not in _CACHE:
        _CACHE["nc"] = _build_program()
    return _CACHE["nc"]


def _split_fp8(a, axis=0):
    """a (f32) -> (hi, lo) fp8e4m3 stacked on `axis` with hi + lo ~= a."""
    import ml_dtypes

    hi = a.astype(ml_dtypes.float8_e4m3)
    lo = (a - hi.astype(np.float32)).astype(ml_dtypes.float8_e4m3)
    return np.ascontiguousarray(np.stack([hi, lo], axis=axis))


def kernel(x, Wq, bq, Wk, bk, Wv, bv, Wo, bo):
    import ml_dtypes
    from concourse.bass_utils import run_bass_kernel_spmd

    nc = get_program()
    x = np.ascontiguousarray(np.asarray(x, dtype=np.float32))
    Wq = np.asarray(Wq, dtype=np.float32)
    Wk = np.asarray(Wk, dtype=np.float32)
    Wv = np.asarray(Wv, dtype=np.float32)
    ws = {
        "Wq8": _split_fp8(WSCL * Wq),
        "Wk8": _split_fp8(WSCL * Wk),
        "Wv8": _split_fp8(WSCL * Wv),
        "Wo": np.ascontiguousarray(np.asarray(Wo, dtype=np.float32).astype(ml_dtypes.bfloat16)),
        "bq": WSCL * np.asarray(bq, dtype=np.float32),
        "bk": WSCL * np.asarray(bk, dtype=np.float32),
    }
    bv = np.asarray(bv, dtype=np.float64)
    bo = np.asarray(bo, dtype=np.float64)
    in_maps = [
        dict(ws, xT8=_split_fp8(x[b].transpose(1, 2, 0), axis=1))
        for b in range(N_CORES)
    ]
    res = run_bass_kernel_spmd(nc, in_maps, list(range(N_CORES)))
    outp = np.stack([res.results[b]["out"] for b in range(N_CORES)], axis=0)
    # bv and bo fold into a constant output row: softmax rows sum to 1, so
    # attention(v + bv) = attention(v) + bv, and (o + bv) @ Wo + bo adds
    # (bv @ Wo + bo) to every output row.
    corr = bv @ np.asarray(Wo, dtype=np.float64) + bo
    if np.any(corr):
        outp = (outp.astype(np.float64) + corr[None, None, None, :]).astype(
            np.float32)
    return outp


# revision 23
# speedup vs baseline: 1.4068x; 1.0588x over previous
"""Trainium2 Bass kernel for nn_MultiHeadedAttention_41566693491186.

Three dual-score MHAs over the streams packed in x[:, :, 0:3, :], with shared
Wq/Wk/Wv/Wo. Data-parallel over batch B=8: one batch element per NeuronCore.

v3 design:
  - Host precomputes xT = x^T per stream and splits xT and 32*W{q,k,v} into
    fp8e4m3 (hi, lo) pairs: A ~= hi + lo with ~0.15% residual.  The nine
    input projections run as fp8 DoubleRow matmuls (2 k-tiles per pass)
    keeping hi*hi + hi*lo + lo*hi cross terms: 12 DR matmuls per [128,512]
    output tile vs 16 f32r matmuls, at near-bf16 accuracy.
  - The 32x weight scale cancels exactly: exp scale becomes 2^-14 (q and k
    both carry 32x), and the v ones-column is 32.0 so softmax denominators
    scale with the numerators.
  - All attention-side tensors (qT/kT/v spills, qcat/kcat/vext, p, OT) are
    bf16: same 1 cyc/row PE cost as f32r, half the DMA/SBUF.
  - Softmax denominators: exact DVE reciprocal + f32r K=1 broadcast matmul
    (1 cyc/row), normalize with one DVE mul into OT.
  - QK^T / PV / out-projection stay f32r-grade (bf16 inputs, f32 PSUM).

Per-core plan:
  P1  projections (interleaved with attention below):
        qT[s] = (32 x_s Wq)^T, kT[s] = (32 x_s Wk)^T  (W-stationary, [j, L])
        v[s]  =  32 x_s Wv    (x-stationary, out [L, j], interleaved with a
                               32.0 column per head for the denominators)
  P2  per (mha, head): S^T = kcat^T-chunks x qcat -> exp (ACT, scale 2^-14,
      bf16 out) -> PV accumulate o^T[d, q] + sums row -> DVE recip ->
      f32r broadcast -> DVE mul into OT (bf16).
  P3  out = OT^T @ Wo + bo  (OT-stationary, out [q, d_model]) -> DRAM.
"""

import sys

if "/opt/trn_rl_repo" not in sys.path:
    sys.path.insert(0, "/opt/trn_rl_repo")

import numpy as np

B, L, D = 8, 1024, 1024
H, DH = 16, 64
NCH = 8              # 128-sized chunks along D or L
SCALE = 0.0625 / 1024.0   # (1/sqrt(64)) * 0.5 / (32*32)
WSCL = 32.0
N_CORES = 8
# mha m reads (A, B, V) streams: q1/k1 from A, q2/k2 from B, v from V
MHA_STREAMS = ((1, 2, 0), (0, 2, 1), (0, 1, 2))

_CACHE = {}


def _split_excess_waits(nc, max_waits=1):
    """Stock neuronxcc walrus rejects instructions carrying more than
    `max_waits` semaphore waits; move excess onto same-engine NOPs."""
    import concourse.mybir as mybir

    for f in nc.m.functions:
        for bb in f.blocks:
            out = []
            changed = False
            for inst in bb.instructions:
                si = inst.sync_info
                waits = list(si.on_wait) if (si is not None and si.on_wait) else []
                if len(waits) > max_waits:
                    extra, keep = waits[:-max_waits], waits[-max_waits:]
                    k = 0
                    while extra:
                        chunk, extra = extra[:max_waits], extra[max_waits:]
                        nop = mybir.InstNoOp(
                            name=f"{inst.name}-ws{k}",
                            engine=inst.engine,
                            sync_info=mybir.SyncInfo(on_wait=chunk, on_update=[]),
                        )
                        out.append(nop)
                        k += 1
                    inst.sync_info = mybir.SyncInfo(
                        on_wait=keep,
                        on_update=list(si.on_update) if si.on_update else [],
                    )
                    changed = True
                out.append(inst)
            if changed:
                bb.instructions = out


def _interleave(*seqs):
    """Proportional merge of thunk lists, preserving within-list order."""
    items = []
    for si, seq in enumerate(seqs):
        n = len(seq)
        for i, thunk in enumerate(seq):
            items.append(((i + 0.5) / n, si, i, thunk))
    for _, _, _, t in sorted(items, key=lambda z: (z[0], z[1], z[2])):
        t()


def _build_program(repeat=1):
    import concourse.bass as bass
    import concourse.mybir as mybir
    import concourse.tile as tile

    f32 = mybir.dt.float32
    f32r = mybir.dt.float32r
    bf16 = mybir.dt.bfloat16
    f8 = mybir.dt.float8e4
    DR = mybir.MatmulPerfMode.DoubleRow
    AF = mybir.ActivationFunctionType

    nc = bass.Bass("TRN2", target_bir_lowering=False, debug=False)

    # hi/lo fp8 pairs, packed [2, D, L]: index 0 = hi, 1 = lo
    xT8 = nc.declare_dram_parameter("xT8", [3, 2, D, L], f8, isOutput=False)
    Wq8 = nc.declare_dram_parameter("Wq8", [2, D, D], f8, isOutput=False)
    Wk8 = nc.declare_dram_parameter("Wk8", [2, D, D], f8, isOutput=False)
    Wv8 = nc.declare_dram_parameter("Wv8", [2, D, D], f8, isOutput=False)
    Wo = nc.declare_dram_parameter("Wo", [D, D], bf16, isOutput=False)
    bq = nc.declare_dram_parameter("bq", [D], f32, isOutput=False)
    bk = nc.declare_dram_parameter("bk", [D], f32, isOutput=False)
    out = nc.declare_dram_parameter("out", [L, 3, D], f32, isOutput=True)

    # internal DRAM spill (bf16)
    qT_d = [nc.dram_tensor(f"qT{s}", [D, L], bf16) for s in range(3)]
    kT_d = [nc.dram_tensor(f"kT{s}", [D, L], bf16) for s in range(3)]
    # v: head h data at cols 65h..65h+64, 32.0 column at 65h+64
    v_d = [nc.dram_tensor(f"v{s}", [L, H * 65], bf16) for s in range(3)]

    with tile.TileContext(nc) as tc:
        cstack = []
        cp = tc.alloc_tile_pool(name="const", bufs=1)
        psum = tc.alloc_tile_pool(name="psum", bufs=1, space="PSUM")
        xts = tc.alloc_tile_pool(name="xts", bufs=4)
        cstack += [cp, psum, xts]

        cmisc = cp.tile([128, 96], f32, tag="cmisc", name="cmisc")
        ones64 = cmisc[:, 0:64]
        v32c = cmisc[:, 80:96]
        bq_t = cmisc[:, 64:72]
        bk_t = cmisc[:, 72:80]
        nc.gpsimd.memset(ones64, 1.0)
        nc.gpsimd.memset(v32c, WSCL)
        onesr = cp.tile([1, 64], f32r, tag="onesr", name="onesr")
        nc.vector.tensor_copy(onesr[:], ones64[0:1, :])
        ones_r = onesr
        nc.sync.dma_start(out=bq_t, in_=bq.rearrange("(c p) -> p c", p=128))
        nc.sync.dma_start(out=bk_t, in_=bk.rearrange("(c p) -> p c", p=128))

        # ---------------- xT loads (fp8 hi+lo, host pre-transposed) ---------
        xt_tiles = {}

        def load_xt(s):
            # [128, hilo, c, l] fp8
            xt = xts.tile([128, 2, NCH, L], f8, tag="xts", name=f"xt{s}")
            src = xT8[s].rearrange("t (c p) l -> p t c l", p=128)
            for t in range(2):
                nc.sync.dma_start(out=xt[:, t, 0:4, :], in_=src[:, t, 0:4, :])
                nc.scalar.dma_start(out=xt[:, t, 4:8, :], in_=src[:, t, 4:8, :])
            xt_tiles[s] = xt

        # ---------------- shared pools for P1/P2/P3 ----------------
        wrp = tc.alloc_tile_pool(name="wrp", bufs=1)
        stp = tc.alloc_tile_pool(name="stp", bufs=5)
        qkp = tc.alloc_tile_pool(name="qkp", bufs=3)
        ptp = tc.alloc_tile_pool(name="ptp", bufs=3)
        rbp = tc.alloc_tile_pool(name="rbp", bufs=2)
        cstack += [wrp, stp, qkp, ptp, rbp]

        def load_w8(Wsrc):
            # [128, hilo, c, d] fp8
            w_t = wrp.tile([128, 2, NCH, D], f8, tag="W8", name="W8", bufs=2)
            src = Wsrc.rearrange("t (c p) d -> p t c d", p=128)
            for t in range(2):
                nc.sync.dma_start(out=w_t[:, t, 0:4, :], in_=src[:, t, 0:4, :])
                nc.scalar.dma_start(out=w_t[:, t, 4:8, :], in_=src[:, t, 4:8, :])
            return w_t

        def load_wo(Wsrc):
            w_t = wrp.tile([128, NCH * D], bf16, tag="Wor", name="Wor")
            d3 = w_t[:].rearrange("p (c d) -> p c d", d=D)
            s3 = Wsrc.rearrange("(c p) d -> p c d", p=128)
            nc.sync.dma_start(out=d3[:, 0:4, :], in_=s3[:, 0:4, :])
            nc.scalar.dma_start(out=d3[:, 4:8, :], in_=s3[:, 4:8, :])
            return w_t

        def dr_products(emit, w_t, xt):
            """12 DoubleRow matmuls accumulating hi*hi + hi*lo + lo*hi over
            4 chunk-pairs; emit(lhs_sel, rhs_sel, t, first, last)."""
            combos = ((0, 0), (0, 1), (1, 0))
            n = 0
            for t in range(4):
                for (wi, xi) in combos:
                    n += 1
                    emit(wi, xi, t, n == 1, n == 12)

        def proj_qk_blocks(w_t, b_t, s, outd, xt):
            # out [j, L] = (32 x_s W)^T, one block per jc
            def block(jc):
                def run():
                    st = stp.tile([128, L], bf16, tag="stq", name="st")
                    for lh in range(2):
                        ps = psum.tile([128, 512], f32, tag="pp",
                                       name="pp", bufs=2)

                        def emit(wi, xi, t, first, last, lh=lh, ps=ps):
                            nc.tensor.matmul(
                                ps[:],
                                lhsT=w_t[:, wi, 2 * t:2 * t + 2,
                                         128 * jc:128 * (jc + 1)],
                                rhs=xt[:, xi, 2 * t:2 * t + 2,
                                       512 * lh:512 * (lh + 1)],
                                start=first, stop=last, perf_mode=DR)
                        dr_products(emit, w_t, xt)
                        nc.vector.tensor_scalar_add(
                            st[:, 512 * lh:512 * (lh + 1)], ps[:],
                            b_t[:, jc:jc + 1])
                    nc.scalar.dma_start(
                        out=outd[s][128 * jc:128 * (jc + 1), :], in_=st[:])
                return run
            return [block(jc) for jc in range(NCH)]

        def proj_v_blocks(w_t, s, xt):
            # out [L, j] with interleaved 32.0 columns, one block per lc
            def block(lc):
                def run():
                    for jh in range(2):
                        ps = psum.tile([128, 512], f32, tag="pp",
                                       name="pp", bufs=2)

                        def emit(wi, xi, t, first, last, jh=jh, ps=ps):
                            nc.tensor.matmul(
                                ps[:],
                                lhsT=xt[:, xi, 2 * t:2 * t + 2,
                                        128 * lc:128 * (lc + 1)],
                                rhs=w_t[:, wi, 2 * t:2 * t + 2,
                                        512 * jh:512 * (jh + 1)],
                                start=first, stop=last, perf_mode=DR)
                        dr_products(emit, w_t, xt)
                        vst = stp.tile([128, 8 * 65], bf16, tag="stv", name="vst")
                        r = vst[:].rearrange("p (h w) -> p h w", w=65)
                        q3 = ps[:].rearrange("p (h w) -> p h w", w=64)
                        nc.vector.tensor_copy(r[:, :, 0:64], q3)
                        nc.vector.tensor_copy(
                            r[:, :, 64:65].squeeze(2), v32c[:, 0:8])
                        nc.scalar.dma_start(
                            out=v_d[s][128 * lc:128 * (lc + 1),
                                       8 * 65 * jh:8 * 65 * (jh + 1)],
                            in_=vst[:])
                return run
            return [block(lc) for lc in range(NCH)]

        def proj_v_units(w_t, s, xt):
            # one unit per (lc, jh) half-block
            def unit(lc, jh):
                def run():
                    ps = psum.tile([128, 512], f32, tag="pp",
                                   name="pp", bufs=2)

                    def emit(wi, xi, t, first, last):
                        nc.tensor.matmul(
                            ps[:],
                            lhsT=xt[:, xi, 2 * t:2 * t + 2,
                                    128 * lc:128 * (lc + 1)],
                            rhs=w_t[:, wi, 2 * t:2 * t + 2,
                                    512 * jh:512 * (jh + 1)],
                            start=first, stop=last, perf_mode=DR)
                    dr_products(emit, w_t, xt)
                    vst = stp.tile([128, 8 * 65], bf16, tag="stv", name="vst")
                    r = vst[:].rearrange("p (h w) -> p h w", w=65)
                    q3 = ps[:].rearrange("p (h w) -> p h w", w=64)
                    nc.vector.tensor_copy(r[:, :, 0:64], q3)
                    nc.vector.tensor_copy(
                        r[:, :, 64:65].squeeze(2), v32c[:, 0:8])
                    nc.scalar.dma_start(
                        out=v_d[s][128 * lc:128 * (lc + 1),
                                   8 * 65 * jh:8 * 65 * (jh + 1)],
                        in_=vst[:])
                return run
            return [unit(lc, jh) for lc in range(NCH) for jh in range(2)]

        def attention_blocks(m, OT):
            sa, sb, sv = MHA_STREAMS[m]
            pend = {}

            def finalize(h, oc):
                po, co = 64 * (h % 2), (h // 2) * L
                rb = rbp.tile([1, L], f32, tag="rb", name="rb")
                rb_r = rbp.tile([1, L], f32r, tag="rbr", name="rbr")
                nc.vector.reciprocal(rb[0:1, :], oc[64:65, :])
                nc.vector.tensor_copy(rb_r[0:1, :], rb[0:1, :])
                for qh in range(2):
                    rb_ps = psum.tile([64, 512], f32, tag="pp",
                                      name="rb_ps", bufs=2)
                    nc.tensor.matmul(
                        rb_ps[:], lhsT=ones_r[0:1, :],
                        rhs=rb_r[0:1, 512 * qh:512 * (qh + 1)],
                        start=True, stop=True)
                    nc.vector.tensor_mul(
                        OT[po:po + 64, co + 512 * qh:co + 512 * (qh + 1)],
                        oc[0:64, 512 * qh:512 * (qh + 1)], rb_ps[:])

            def head_units(h):
                """Chunk-granular thunks: [load+QK0, (QK1,PV0), ...,
                (QK7,PV6), (PV7,copies,finalize-prev)]."""
                st = {}

                def qk(c):
                    s_ps = psum.tile([128, L], f32, tag="scr", name="scr",
                                     bufs=2)
                    for qh in range(2):
                        nc.tensor.matmul(
                            s_ps[:, 512 * qh:512 * (qh + 1)],
                            lhsT=st["kcat"][:, 128 * c:128 * (c + 1)],
                            rhs=st["qcat"][:, 512 * qh:512 * (qh + 1)],
                            start=True, stop=True)
                    p_sb = ptp.tile([128, L], bf16, tag="p_sb", name="p_sb")
                    nc.scalar.activation(p_sb[:], s_ps[:], AF.Exp, scale=SCALE)
                    st[c] = p_sb

                def pv(c):
                    for qh in range(2):
                        nc.tensor.matmul(
                            st["o_ps"][qh][0:65, :],
                            lhsT=st["vext"][:, 65 * c:65 * (c + 1)],
                            rhs=st[c][:, 512 * qh:512 * (qh + 1)],
                            start=(c == 0), stop=(c == NCH - 1))
                    del st[c]

                def u_load():
                    qcat = qkp.tile([128, L], bf16, tag="qcat", name="qcat",
                                    bufs=4)
                    kcat = qkp.tile([128, L], bf16, tag="kcat", name="kcat",
                                    bufs=4)
                    nc.sync.dma_start(
                        out=qcat[0:64, :], in_=qT_d[sa][64 * h:64 * h + 64, :])
                    nc.sync.dma_start(
                        out=qcat[64:128, :], in_=qT_d[sb][64 * h:64 * h + 64, :])
                    nc.sync.dma_start(
                        out=kcat[0:64, :], in_=kT_d[sa][64 * h:64 * h + 64, :])
                    nc.sync.dma_start(
                        out=kcat[64:128, :], in_=kT_d[sb][64 * h:64 * h + 64, :])
                    vext = qkp.tile([128, NCH * 65], bf16, tag="vext",
                                    name="vext", bufs=3)
                    vsrc = v_d[sv].rearrange("(c p) w -> p c w", p=128)
                    nc.sync.dma_start(
                        out=vext[:].rearrange("p (c w) -> p c w", w=65),
                        in_=vsrc[:, :, 65 * h:65 * (h + 1)])
                    st["qcat"], st["kcat"], st["vext"] = qcat, kcat, vext
                    st["o_ps"] = [psum.tile([65, 512], f32, tag="ops",
                                            name="ops", bufs=2)
                                  for _ in range(2)]
                    qk(0)

                def u_mid(c):
                    qk(c)
                    pv(c - 1)

                def u_tail():
                    pv(NCH - 1)
                    oc = stp.tile([65, L], f32, tag="stoc", name="oc")
                    for qh in range(2):
                        nc.vector.tensor_copy(
                            oc[:, 512 * qh:512 * (qh + 1)],
                            st["o_ps"][qh][:])
                    if pend:
                        (ph, poc), = pend.items()
                        finalize(ph, poc)
                        pend.clear()
                    pend[h] = oc

                return ([u_load] + [lambda c=c: u_mid(c)
                                    for c in range(1, NCH)] + [u_tail])

            def tail():
                (ph, poc), = pend.items()
                finalize(ph, poc)
                pend.clear()

            units = []
            for h in range(H):
                units += head_units(h)
            units.append(tail)
            return units

        def oproj_units(m, OT, wo_t):
            # one unit per (qc, dh) half-block; DMA issued on the dh=1 unit
            osts = {}

            def unit(qc, dh):
                def run():
                    if dh == 0:
                        osts[qc] = stp.tile([128, L], f32, tag="stoc",
                                            name="ost")
                    ost = osts[qc]
                    op_ps = psum.tile([128, 512], f32, tag="pp",
                                      name="pp", bufs=2)
                    for c in range(NCH):
                        nc.tensor.matmul(
                            op_ps[:],
                            lhsT=OT[:, L * c + 128 * qc:L * c + 128 * (qc + 1)],
                            rhs=wo_t[:, D * c + 512 * dh:D * c + 512 * (dh + 1)],
                            start=(c == 0), stop=(c == NCH - 1))
                    nc.vector.tensor_copy(
                        ost[:, 512 * dh:512 * (dh + 1)], op_ps[:])
                    if dh == 1:
                        nc.scalar.dma_start(
                            out=out[128 * qc:128 * (qc + 1), m, :], in_=ost[:])
                        del osts[qc]
                return run
            return [unit(qc, dh) for qc in range(NCH) for dh in range(2)]

        for _rep in range(repeat):
            # ---------------- emission schedule ----------------
            OTs = {}

            def mk_ot(m):
                OTs[m] = xts.tile([128, NCH * L], bf16, tag="xts", name="OT")

            wq_t = load_w8(Wq8)
            load_xt(1)
            load_xt(2)
            load_xt(0)
            for b in proj_qk_blocks(wq_t, bq_t, 1, qT_d, xt_tiles[1]):
                b()
            wk_t = load_w8(Wk8)
            for s in (2, 0):
                for b in proj_qk_blocks(wq_t, bq_t, s, qT_d, xt_tiles[s]):
                    b()
            for b in proj_qk_blocks(wk_t, bk_t, 1, kT_d, xt_tiles[1]):
                b()
            wv_t = load_w8(Wv8)
            for s in (2, 0):
                for b in proj_qk_blocks(wk_t, bk_t, s, kT_d, xt_tiles[s]):
                    b()
            for b in proj_v_blocks(wv_t, 0, xt_tiles[0]):
                b()

            # A0 || (v1, v2): xt1/xt2 and Wv still resident
            mk_ot(0)

            _interleave(attention_blocks(0, OTs[0]),
                        proj_v_units(wv_t, 1, xt_tiles[1])
                        + proj_v_units(wv_t, 2, xt_tiles[2]))

            # A1 || (load Wo, oproj 0)
            mk_ot(1)
            wo_state = {}

            def o0_blocks():
                blocks = []

                def loadwo():
                    wo_state["w"] = load_wo(Wo)
                blocks.append(loadwo)

                def get_units():
                    if "u0" not in wo_state:
                        wo_state["u0"] = oproj_units(0, OTs[0], wo_state["w"])
                    return wo_state["u0"]
                for u in range(16):
                    blocks.append(lambda u=u: get_units()[u]())
                return blocks

            _interleave(attention_blocks(1, OTs[1]), o0_blocks())

            # A2 || oproj 1
            mk_ot(2)
            o1_units = oproj_units(1, OTs[1], wo_state["w"])
            _interleave(attention_blocks(2, OTs[2]),
                        [lambda u=u: o1_units[u]() for u in range(16)])

            for u in oproj_units(2, OTs[2], wo_state["w"]):
                u()

        for p in reversed(cstack):
            p.release()

    _split_excess_waits(nc, max_waits=1)
    return nc


def get_program():
    if "nc" not in _CACHE:
        _CACHE["nc"] = _build_program()
    return _CACHE["nc"]


def _split_fp8(a, axis=0):
    """a (f32) -> (hi, lo) fp8e4m3 stacked on `axis` with hi + lo ~= a."""
    import ml_dtypes

    hi = a.astype(ml_dtypes.float8_e4m3)
    lo = (a - hi.astype(np.float32)).astype(ml_dtypes.float8_e4m3)
    return np.ascontiguousarray(np.stack([hi, lo], axis=axis))


def kernel(x, Wq, bq, Wk, bk, Wv, bv, Wo, bo):
    import ml_dtypes
    from concourse.bass_utils import run_bass_kernel_spmd

    nc = get_program()
    x = np.ascontiguousarray(np.asarray(x, dtype=np.float32))
    Wq = np.asarray(Wq, dtype=np.float32)
    Wk = np.asarray(Wk, dtype=np.float32)
    Wv = np.asarray(Wv, dtype=np.float32)
    ws = {
        "Wq8": _split_fp8(WSCL * Wq),
        "Wk8": _split_fp8(WSCL * Wk),
        "Wv8": _split_fp8(WSCL * Wv),
        "Wo": np.ascontiguousarray(np.asarray(Wo, dtype=np.float32).astype(ml_dtypes.bfloat16)),
        "bq": WSCL * np.asarray(bq, dtype=np.float32),
        "bk": WSCL * np.asarray(bk, dtype=np.float32),
    }
    bv = np.asarray(bv, dtype=np.float64)
    bo = np.asarray(bo, dtype=np.float64)
    in_maps = [
        dict(ws, xT8=_split_fp8(x[b].transpose(1, 2, 0), axis=1))
        for b in range(N_CORES)
    ]
    res = run_bass_kernel_spmd(nc, in_maps, list(range(N_CORES)))
    outp = np.stack([res.results[b]["out"] for b in range(N_CORES)], axis=0)
    # bv and bo fold into a constant output row: softmax rows sum to 1, so
    # attention(v + bv) = attention(v) + bv, and (o + bv) @ Wo + bo adds
    # (bv @ Wo + bo) to every output row.
    corr = bv @ np.asarray(Wo, dtype=np.float64) + bo
    if np.any(corr):
        outp = (outp.astype(np.float64) + corr[None, None, None, :]).astype(
            np.float32)
    return outp
